# revision 21
# baseline (speedup 1.0000x reference)
"""Multi-head attention (B=4, S=2048, E=1024, H=16) on 8 TRN2 NeuronCores.

Sharding: batch x head-group tensor parallel -- core c = 2*b + hg handles
batch b and heads hg*8 .. hg*8+7 for ALL 2048 queries.  Q/K/V projections
are column-split by head (each core projects only its 8 heads); the output
projection is row-split (each core contracts its 512 E-rows of W_out) and
produces a partial [E, S] output that the HOST sums across the core pair
while unsharding (the "all-reduce" of the sharding hint, done on host).

Per-core kernel:
  - Q^T/K^T projections (bf16 matmul, fp32 PSUM) evacuated with fused
    bias-add + fp8e4 quantization (DVE tensor_scalar_add, fp8 out).
  - scores via fp8 DoubleRow matmuls: contraction d=64 fed as
    [64 part, 2(dup, stride 0), N]; the duplicated group doubles the
    result and the exp activation scale absorbs the factor 2.
    Cost: 0.5 cycles/row (vs 1.0 bf16).
  - exp on ScalarE (the bottleneck engine, ~266us busy): one [128, 1024]
    activation per key-tile j, reading two adjacent 512-wide slots of a
    manual 4-slot PSUM ring (slots 2j%4, 2j%4+1 -> flat AP; the two slot
    pairs double-buffer).  Scores for j+1 are emitted BEFORE the PV
    matmuls of j so the exp stream never waits on PE's in-order queue.
  - PV in the FLIPPED orientation: out[q=128, 65] = P_tile.T @ [V | ones]
    (all 128 output partitions vs 65 the naive way); the ones column is
    the softmax denominator per query row.  The 8 per-qt accumulators
    live in one [128, 8, 128] tile (qt stride 512B -> no bank crossing);
    PSUM start=True zero-fills a whole 2KB region, so only the first
    matmul touching each bank uses start=True and the rest rely on the
    pending-zero overwrite semantics (no memset needed).
  - normalization: per-partition reciprocal of the denominator column +
    tensor_scalar multiply -> O in [q, d]; PE-transpose (identity
    matmul) back to O^T for the out projection.

Schedule: 2 query-half phase groups x 8 heads x 16 key tiles (j).
Per j-step the PE also runs one or two small interleaved chunks: V
projection for the NEXT head (just-in-time, ~240ns each), K/Q
projection half-chunks (~850ns), O^T transposes, and the first half's
output projection (during the second half).  Only the second half's
output projection is a serial tail.
"""

import sys

if "/opt/trn_rl_repo" not in sys.path:
    sys.path.insert(0, "/opt/trn_rl_repo")

import numpy as np
import ml_dtypes

B, S, E, H = 4, 2048, 1024, 16
P = 128
HD = 64           # head dim
NH = 8            # heads per core
DT = 4            # d-tiles (head pairs) per core
ET = E // P       # 8 e-tiles (contraction for projections)
ST = S // P       # 16 key tiles
N_CORES = 8
QH = S // 2       # query half width (1024)
SCALE = 1.0 / float(np.sqrt(HD))

_BF16 = ml_dtypes.bfloat16

_cached = None


def _build():
    import concourse.bass as bass
    import concourse.tile as tile
    import concourse.mybir as mybir
    from concourse import bacc

    dt = mybir.dt
    nc = bacc.Bacc("TRN2", target_bir_lowering=False, debug=False)

    xt_d = nc.dram_tensor("xt", [E, S], dt.bfloat16, kind="ExternalInput").ap()
    wq_d = nc.dram_tensor("wq", [E, 512], dt.bfloat16, kind="ExternalInput").ap()
    wk_d = nc.dram_tensor("wk", [E, 512], dt.bfloat16, kind="ExternalInput").ap()
    wv_d = nc.dram_tensor("wv", [E, 512], dt.bfloat16, kind="ExternalInput").ap()
    wo_d = nc.dram_tensor("wo", [512, E], dt.bfloat16, kind="ExternalInput").ap()
    bq_d = nc.dram_tensor("bq", [P, DT], dt.float32, kind="ExternalInput").ap()
    bk_d = nc.dram_tensor("bk", [P, DT], dt.float32, kind="ExternalInput").ap()
    bv_d = nc.dram_tensor("bv", [1, 512], dt.bfloat16, kind="ExternalInput").ap()
    bo_d = nc.dram_tensor("bo", [P, ET], dt.float32, kind="ExternalInput").ap()
    iden_d = nc.dram_tensor("iden", [P, P], dt.bfloat16, kind="ExternalInput").ap()
    out_d = nc.dram_tensor("out", [E, S], dt.float32, kind="ExternalOutput").ap()

    DR = mybir.MatmulPerfMode.DoubleRow

    with tile.TileContext(nc) as tc:
        with (
            tc.tile_pool(name="const", bufs=1) as cpool,
            tc.tile_pool(name="acts", bufs=1) as apool,
            tc.tile_pool(name="pp", bufs=3) as ppool,        # P (exp out)
            tc.tile_pool(name="oqp", bufs=2) as oqpool,      # O [q, dd] staging
            tc.tile_pool(name="recp", bufs=2) as recpool,    # reciprocals
            tc.tile_pool(name="outs", bufs=4) as outpool,    # out staging
            tc.tile_pool(name="pssc", bufs=1, space="PSUM") as scpool,   # 4 banks
            tc.tile_pool(name="pspv", bufs=1, space="PSUM") as pvpool,   # 2 banks
            tc.tile_pool(name="pspj", bufs=1, space="PSUM") as pjpool,   # 1 bank
            tc.tile_pool(name="pstp", bufs=1, space="PSUM") as tppool,   # 1 bank
        ):
            # ---------------- constants / inputs -----------------------
            xt = cpool.tile([P, ET, S], dt.bfloat16)
            wq = cpool.tile([P, ET, 512], dt.bfloat16)
            wk = cpool.tile([P, ET, 512], dt.bfloat16)
            wv = cpool.tile([P, ET, 512], dt.bfloat16)
            wo = cpool.tile([P, DT, E], dt.bfloat16)
            bq = cpool.tile([P, DT], dt.float32)
            bk = cpool.tile([P, DT], dt.float32)
            bv = cpool.tile([1, 512], dt.bfloat16)
            bo = cpool.tile([P, ET], dt.float32)
            iden = cpool.tile([P, P], dt.bfloat16)
            ones1 = cpool.tile([1, P], dt.bfloat16)

            # activations
            qt8 = apool.tile([P, DT, S], dt.float8e4)   # Q^T (bias+fp8)
            kt8 = apool.tile([P, DT, S], dt.float8e4)   # K^T (bias+fp8)
            va = apool.tile([P, ST, NH, HD + 1], dt.bfloat16)  # V | ones
            scb = apool.tile([P, DT, S], dt.bfloat16)   # O^T (normalized)

            # long-lived PSUM tiles (sub-region dependency tracking)
            ring = scpool.tile([P, 4, 512], dt.float32, tag="sc", name="ring")
            pv = pvpool.tile([P, 8, P], dt.float32, tag="pv", name="pv")

            nc.sync.dma_start(wk[:, :, :], wk_d.rearrange("(eo p) c -> p eo c", p=P))
            for e in range(ET):
                nc.sync.dma_start(xt[:, e, :], xt_d[e * P : (e + 1) * P, :])
            nc.sync.dma_start(wq[:, :, :], wq_d.rearrange("(eo p) c -> p eo c", p=P))
            nc.sync.dma_start(wv[:, :, :], wv_d.rearrange("(eo p) c -> p eo c", p=P))
            nc.sync.dma_start(bq[:], bq_d)
            nc.sync.dma_start(bk[:], bk_d)
            nc.sync.dma_start(bv[:], bv_d)
            nc.sync.dma_start(iden[:], iden_d)
            nc.sync.dma_start(wo[:], wo_d.rearrange("(eo p) c -> p eo c", p=P))
            nc.sync.dma_start(bo[:], bo_d)
            nc.gpsimd.memset(ones1[:], 1.0)
            nc.gpsimd.memset(va[:, :, :, HD : HD + 1], 1.0)

            # ---------------- small-chunk emitters ----------------------
            # kq/outproj chunks come as (partA, partB) sharing one psum tile;
            # with a single-buffer proj pool they are also adjacent-safe.

            def kq_halves(which, t, c):
                w_, b_, dst = (wk, bk, kt8) if which == "k" else (wq, bq, qt8)
                holder = {}

                def part0():
                    ps = pjpool.tile(
                        [P, 512], dt.float32, tag="pj", name=f"{which}{t}{c}"
                    )
                    holder[0] = ps
                    for e in range(4):
                        nc.tensor.matmul(
                            ps[:],
                            w_[:, e, t * P : (t + 1) * P],
                            xt[:, e, c * 512 : (c + 1) * 512],
                            start=(e == 0),
                            stop=False,
                        )

                def part1():
                    ps = holder[0]
                    for e in range(4, ET):
                        nc.tensor.matmul(
                            ps[:],
                            w_[:, e, t * P : (t + 1) * P],
                            xt[:, e, c * 512 : (c + 1) * 512],
                            start=False,
                            stop=(e == ET - 1),
                        )
                    nc.vector.tensor_scalar_add(
                        dst[:, t, c * 512 : (c + 1) * 512], ps[:], b_[:, t : t + 1]
                    )

                return [part0, part1]

            def v_chunk(h, st):
                """V rows for (head h, key tile st): [128 keys, 64] + bias.
                Uses the transpose-psum bank (tiny tiles) to stay off the
                kq/outproj pipeline."""
                def go():
                    ps = tppool.tile([P, P], dt.float32, tag="vps", name=f"v{h}{st}")
                    for e in range(ET):
                        nc.tensor.matmul(
                            ps[:, 0:HD],
                            xt[:, e, st * P : (st + 1) * P],
                            wv[:, e, h * HD : (h + 1) * HD],
                            start=(e == 0),
                            stop=False,
                        )
                    nc.tensor.matmul(
                        ps[:, 0:HD],
                        ones1[0:1, :],
                        bv[0:1, h * HD : (h + 1) * HD],
                        start=False,
                        stop=True,
                    )
                    nc.vector.tensor_copy(va[:, st, h, 0:HD], ps[:, 0:HD])
                return go

            def outproj_halves(eo, qq):
                """Partial out^T tile [128 Eo, 512 q] for query quarter qq."""
                holder = {}
                q0 = qq * 512

                def part0():
                    ps = pjpool.tile([P, 512], dt.float32, tag="pj", name=f"o{eo}{qq}")
                    holder[0] = ps
                    for t in (0, 1):
                        nc.tensor.matmul(
                            ps[:],
                            wo[:, t, eo * P : (eo + 1) * P],
                            scb[:, t, q0 : q0 + 512],
                            start=(t == 0),
                            stop=False,
                        )

                def part1():
                    ps = holder[0]
                    for t in (2, 3):
                        nc.tensor.matmul(
                            ps[:],
                            wo[:, t, eo * P : (eo + 1) * P],
                            scb[:, t, q0 : q0 + 512],
                            start=False,
                            stop=(t == DT - 1),
                        )
                    ot = outpool.tile(
                        [P, 512], dt.float32, tag="ot", name=f"oe{eo}{qq}"
                    )
                    nc.vector.tensor_scalar_add(ot[:], ps[:], bo[:, eo : eo + 1])
                    nc.sync.dma_start(
                        out_d[eo * P : (eo + 1) * P, q0 : q0 + 512], ot[:]
                    )

                return [part0, part1]

            oq_tiles = {}

            def transpose_item(t, qh, qt):
                """oq [q, dd of pair t] -> scb[:, t, ...] via PE transpose.
                Uses the same psum bank as v_chunk (different tag would
                double-book the bank, so share tag/shape via bitcast)."""
                def go():
                    tp = tppool.tile([P, P], dt.float32, tag="vps", name=f"tp{t}{qh}{qt}")
                    tpb = tp[:, 0:HD].bitcast(dt.bfloat16)
                    nc.tensor.transpose(tpb, oq_tiles[(t, qh)][:, qt, :], iden[:])
                    q0 = qh * QH + qt * P
                    nc.vector.tensor_copy(scb[:, t, q0 : q0 + P], tpb)
                return go

            # ---------------- attention phase ---------------------------
            def emit_attention(h, qh, extra):
                """One head, one query half: 16 j-steps, scores pipelined one
                j ahead of the PV wave; consumes thunks from `extra`."""
                t, hp = h // 2, (h % 2) * HD
                q0 = qh * QH

                def scores(j):
                    s = (2 * j) % 4
                    for qc in range(2):
                        nc.tensor.matmul(
                            ring[:, s + qc, :],
                            kt8[hp : hp + HD, t, j * P : (j + 1) * P]
                            .unsqueeze(1)
                            .broadcast_to((HD, 2, P)),
                            qt8[hp : hp + HD, t, q0 + qc * 512 : q0 + (qc + 1) * 512]
                            .unsqueeze(1)
                            .broadcast_to((HD, 2, 512)),
                            start=True,
                            stop=True,
                            perf_mode=DR,
                        )

                scores(0)
                for j in range(ST):
                    if j < ST - 1:
                        scores(j + 1)
                    s = (2 * j) % 4
                    ptile = ppool.tile(
                        [P, 2, 512], dt.bfloat16, tag="p", name=f"p{h}{qh}{j}"
                    )
                    nc.scalar.activation(
                        ptile.rearrange("p a b -> p (a b)"),
                        ring[:, s : s + 2, :].rearrange("p a b -> p (a b)"),
                        mybir.ActivationFunctionType.Exp,
                        scale=SCALE / 2.0,
                    )
                    for qt in range(8):
                        nc.tensor.matmul(
                            pv[:, qt, 0 : HD + 1],
                            ptile[:, qt // 4, (qt % 4) * P : (qt % 4 + 1) * P],
                            va[:, j, h, :],
                            start=(j == 0 and qt % 4 == 0),
                            stop=(j == ST - 1),
                            skip_group_check=True,
                        )
                    if extra:
                        extra.pop(0)()
                    if extra and len(extra) > ST - 1 - j:
                        extra.pop(0)()

            def emit_evac(h, qh):
                t, half = h // 2, h % 2
                if half == 0:
                    oq_tiles[(t, qh)] = oqpool.tile(
                        [P, 8, P], dt.bfloat16, tag="oq", name=f"oq{t}{qh}"
                    )
                oq = oq_tiles[(t, qh)]
                rec = recpool.tile([P, 8], dt.float32, tag="rec", name=f"rc{h}{qh}")
                scr = recpool.tile([P, 8], dt.float32, tag="scr", name=f"sr{h}{qh}")
                nc.vector.reciprocal_approx_accurate(
                    rec[:], pv[:, :, HD : HD + 1].rearrange("p a b -> p (a b)"), scr[:]
                )
                for qt in range(8):
                    nc.vector.tensor_scalar(
                        oq[:, qt, half * HD : (half + 1) * HD],
                        pv[:, qt, 0:HD],
                        rec[:, qt : qt + 1],
                        None,
                        op0=mybir.AluOpType.mult,
                    )

            # ---------------- emission schedule -------------------------
            # Pre-attention prefix: K/Q d-tile 0 for the first half, with
            # V chunks (separate psum bank) filling the proj-evac gaps.
            prefix_kq = (
                kq_halves("k", 0, 0) + kq_halves("q", 0, 0)
                + kq_halves("k", 0, 1) + kq_halves("q", 0, 1)
            )
            prefix_v = [v_chunk(0, st) for st in range(6)]
            for i, fn in enumerate(prefix_kq):
                fn()
                if i % 2 == 1 and prefix_v:
                    prefix_v.pop(0)()
            for fn in prefix_v:
                fn()

            # Per-phase thunk lists.  Slot i is consumed at the end of j-step
            # i (available from step i+1).  V(h, st) is needed by the PV at
            # step st of phase (h, 0) -> slot <= st-1.
            plans = {(h, qh): [] for qh in range(2) for h in range(NH)}
            plans[(0, 0)] = (
                [v_chunk(0, 6), v_chunk(0, 7)]
                + kq_halves("k", 0, 2)
                + [v_chunk(0, 8), v_chunk(0, 9)]
                + kq_halves("k", 0, 3)
                + [v_chunk(0, st) for st in range(10, ST)]
            )
            for h in range(1, NH):
                # own V just-in-time (slots 0..13 = due st-2), first two tiles
                # at the end of the previous head's phase
                plans[(h - 1, 0)].extend([v_chunk(h, 0), v_chunk(h, 1)])
                plans[(h, 0)] = [v_chunk(h, st) for st in range(2, ST)] + plans[(h, 0)]
            # K d-tiles 1..3 (due at head 2t, half 0) + their half-0 Q
            for t2 in range(1, DT):
                for c in range(4):
                    plans[(2 * t2 - 2 + (c % 2), 0)].extend(kq_halves("k", t2, c))
                plans[(2 * t2 - 1, 0)].extend(
                    kq_halves("q", t2, 0) + kq_halves("q", t2, 1)
                )
            # Q for half 1: d-tile 0 late in half 0; d-tiles 1..3 inside half 1
            plans[(NH - 2, 0)].extend(kq_halves("q", 0, 2))
            plans[(NH - 1, 0)].extend(kq_halves("q", 0, 3))
            for t2 in range(1, DT):
                plans[(2 * t2 - 1, 1)].extend(
                    kq_halves("q", t2, 2) + kq_halves("q", t2, 3)
                )
            # outproj of half 0 (query quarters 0/1) runs during half 1
            for i, eo in enumerate(range(ET)):
                plans[(1 + (eo % 4), 1)].extend(
                    outproj_halves(eo, 0) + outproj_halves(eo, 1)
                )
            # transposes of pair t (ready after head 2t+1) -> following phase
            order = [(h, qh) for qh in range(2) for h in range(NH)]
            tail_items = []
            for qh in range(2):
                for t2 in range(DT):
                    items = [transpose_item(t2, qh, qt) for qt in range(8)]
                    src = (2 * t2 + 1, qh)
                    if src == (NH - 1, 1):
                        tail_items.extend(items)
                    else:
                        nxt = order[order.index(src) + 1]
                        plans[nxt].extend(items)

            # Run all phases.
            for h, qh in order:
                emit_attention(h, qh, plans[(h, qh)])
                emit_evac(h, qh)
                for it in plans[(h, qh)]:
                    it()
                plans[(h, qh)] = []

            # Tail: last pair's transposes + half 1's outproj.  The scores
            # ring is idle now -- use its 4 slots as rotating outproj psum
            # so fills and evacs overlap (the single pj buffer would
            # serialize ~16 chunks).
            for it in tail_items:
                it()
            ci = 0
            for eo in range(ET):
                for qq in (2, 3):
                    ps = ring[:, ci % 4, :]
                    ci += 1
                    q0 = qq * 512
                    for t in range(DT):
                        nc.tensor.matmul(
                            ps,
                            wo[:, t, eo * P : (eo + 1) * P],
                            scb[:, t, q0 : q0 + 512],
                            start=(t == 0),
                            stop=(t == DT - 1),
                        )
                    ot = outpool.tile(
                        [P, 512], dt.float32, tag="ot", name=f"ot{eo}{qq}"
                    )
                    nc.vector.tensor_scalar_add(ot[:], ps, bo[:, eo : eo + 1])
                    nc.sync.dma_start(
                        out_d[eo * P : (eo + 1) * P, q0 : q0 + 512], ot[:]
                    )

    nc.compile()
    return nc


def _prep_inputs(x, W_qkv, b_qkv, W_out, b_out):
    """Host-side sharding + layout prep. Returns per-core input maps."""
    w = W_qkv.reshape(E, H, 3, HD)
    b3 = b_qkv.reshape(H, 3, HD)
    iden = np.eye(P, dtype=np.float32).astype(_BF16)

    in_maps = []
    for core in range(N_CORES):
        b, hg = core // 2, core % 2
        hs = slice(hg * NH, (hg + 1) * NH)
        xt = np.ascontiguousarray(x[b].T).astype(_BF16)           # [E, S]
        wq = np.ascontiguousarray(w[:, hs, 0, :].reshape(E, 512)).astype(_BF16)
        wk = np.ascontiguousarray(w[:, hs, 1, :].reshape(E, 512)).astype(_BF16)
        wv = np.ascontiguousarray(w[:, hs, 2, :].reshape(E, 512)).astype(_BF16)
        wo = np.ascontiguousarray(W_out[hg * 512 : (hg + 1) * 512, :]).astype(_BF16)
        bq = np.ascontiguousarray(b3[hs, 0, :].reshape(DT, P).T).astype(np.float32)
        bk = np.ascontiguousarray(b3[hs, 1, :].reshape(DT, P).T).astype(np.float32)
        bv = np.ascontiguousarray(b3[hs, 2, :].reshape(1, 512)).astype(_BF16)
        bo = (np.ascontiguousarray(b_out.reshape(ET, P).T) * (1.0 if hg == 0 else 0.0)).astype(np.float32)
        in_maps.append(
            {
                "xt": xt,
                "wq": wq,
                "wk": wk,
                "wv": wv,
                "wo": wo,
                "bq": bq,
                "bk": bk,
                "bv": bv,
                "bo": bo,
                "iden": iden,
            }
        )
    return in_maps


def run_raw(x, W_qkv, b_qkv, W_out, b_out, trace=False, **kw):
    """Run on hardware; returns (full_output [B,S,E] f32, BassKernelResults)."""
    global _cached
    from concourse.bass_utils import run_bass_kernel_spmd

    if _cached is None:
        _cached = _build()
    nc = _cached
    in_maps = _prep_inputs(
        np.asarray(x), np.asarray(W_qkv), np.asarray(b_qkv),
        np.asarray(W_out), np.asarray(b_out),
    )
    res = run_bass_kernel_spmd(
        nc, in_maps, core_ids=list(range(N_CORES)), trace=trace, **kw
    )
    out = np.empty((B, S, E), dtype=np.float32)
    for b in range(B):
        acc = np.asarray(res.results[2 * b]["out"]) + np.asarray(
            res.results[2 * b + 1]["out"]
        )
        out[b] = acc.T
    return out, res


def kernel(x, W_qkv, b_qkv, W_out, b_out):
    out, _ = run_raw(x, W_qkv, b_qkv, W_out, b_out, trace=False)
    return out


# revision 30
# speedup vs baseline: 1.1090x; 1.1090x over previous
"""Multi-head attention (B=4, S=2048, E=1024, H=16) on 8 TRN2 NeuronCores.

Sharding: batch x head-group tensor parallel -- core c = 2*b + hg handles
batch b and heads hg*8 .. hg*8+7 for ALL 2048 queries.  Q/K/V projections
are column-split by head (each core projects only its 8 heads); the output
projection is row-split (each core contracts its 512 E-rows of W_out) and
produces a partial [E, S] output that the HOST sums across the core pair
while unsharding (the "all-reduce" of the sharding hint, done on host).

Per-core kernel:
  - Q^T/K^T projections (bf16 matmul, fp32 PSUM) evacuated with fused
    bias-add + fp8e4 quantization (DVE tensor_scalar_add, fp8 out).
  - scores via fp8 DoubleRow matmuls: contraction d=64 fed as
    [64 part, 2(dup, stride 0), N]; the duplicated group doubles the
    result and the exp activation scale absorbs the factor 2.
    Cost: 0.5 cycles/row (vs 1.0 bf16).
  - exp on ScalarE (the bottleneck engine, ~266us busy): one [128, 1024]
    activation per key-tile j, reading two adjacent 512-wide slots of a
    manual 4-slot PSUM ring (slots 2j%4, 2j%4+1 -> flat AP; the two slot
    pairs double-buffer).  Scores for j+1 are emitted BEFORE the PV
    matmuls of j so the exp stream never waits on PE's in-order queue.
  - PV in the FLIPPED orientation: out[q=128, 65] = P_tile.T @ [V | ones]
    (all 128 output partitions vs 65 the naive way); the ones column is
    the softmax denominator per query row.  The 8 per-qt accumulators
    live in one [128, 8, 128] tile (qt stride 512B -> no bank crossing);
    PSUM start=True zero-fills a whole 2KB region, so only the first
    matmul touching each bank uses start=True and the rest rely on the
    pending-zero overwrite semantics (no memset needed).
  - normalization: per-partition reciprocal of the denominator column +
    tensor_scalar multiply -> O in [q, d]; PE-transpose (identity
    matmul) back to O^T for the out projection.

Schedule: 2 query-half phase groups x 8 heads x 16 key tiles (j).
Per j-step the PE also runs one or two small interleaved chunks: V
projection for the NEXT head (just-in-time, ~240ns each), K/Q
projection half-chunks (~850ns), O^T transposes, and the first half's
output projection (during the second half).  Only the second half's
output projection is a serial tail.
"""

import sys

if "/opt/trn_rl_repo" not in sys.path:
    sys.path.insert(0, "/opt/trn_rl_repo")

import numpy as np
import ml_dtypes

B, S, E, H = 4, 2048, 1024, 16
P = 128
HD = 64           # head dim
NH = 8            # heads per core
DT = 4            # d-tiles (head pairs) per core
ET = E // P       # 8 e-tiles (contraction for projections)
ST = S // P       # 16 key tiles
N_CORES = 8
QH = S // 2       # query half width (1024)
SCALE = 1.0 / float(np.sqrt(HD))

_BF16 = ml_dtypes.bfloat16

_cached = None


def _build():
    import concourse.bass as bass
    import concourse.tile as tile
    import concourse.mybir as mybir
    from concourse import bacc

    dt = mybir.dt
    nc = bacc.Bacc("TRN2", target_bir_lowering=False, debug=False)

    xt_d = nc.dram_tensor("xt", [E, S], dt.bfloat16, kind="ExternalInput").ap()
    wq_d = nc.dram_tensor("wq", [E, 512], dt.bfloat16, kind="ExternalInput").ap()
    wk_d = nc.dram_tensor("wk", [E, 512], dt.bfloat16, kind="ExternalInput").ap()
    wv_d = nc.dram_tensor("wv", [E, 512], dt.bfloat16, kind="ExternalInput").ap()
    wo_d = nc.dram_tensor("wo", [512, E], dt.bfloat16, kind="ExternalInput").ap()
    bq_d = nc.dram_tensor("bq", [P, DT], dt.float32, kind="ExternalInput").ap()
    bk_d = nc.dram_tensor("bk", [P, DT], dt.float32, kind="ExternalInput").ap()
    bv_d = nc.dram_tensor("bv", [1, 512], dt.bfloat16, kind="ExternalInput").ap()
    bo_d = nc.dram_tensor("bo", [P, ET], dt.float32, kind="ExternalInput").ap()
    iden_d = nc.dram_tensor("iden", [P, P], dt.bfloat16, kind="ExternalInput").ap()
    out_d = nc.dram_tensor("out", [E, S], dt.float32, kind="ExternalOutput").ap()

    DR = mybir.MatmulPerfMode.DoubleRow

    with tile.TileContext(nc) as tc:
        with (
            tc.tile_pool(name="const", bufs=1) as cpool,
            tc.tile_pool(name="acts", bufs=1) as apool,
            tc.tile_pool(name="pp", bufs=3) as ppool,        # P (exp out)
            tc.tile_pool(name="oqp", bufs=2) as oqpool,      # O [q, dd] staging
            tc.tile_pool(name="recp", bufs=2) as recpool,    # reciprocals
            tc.tile_pool(name="outs", bufs=8) as outpool,    # out staging
            tc.tile_pool(name="pssc", bufs=1, space="PSUM") as scpool,   # 4 banks
            tc.tile_pool(name="pspv", bufs=1, space="PSUM") as pvpool,   # 2 banks
            tc.tile_pool(name="pspj", bufs=1, space="PSUM") as pjpool,   # 1 bank
            tc.tile_pool(name="pstp", bufs=1, space="PSUM") as tppool,   # 1 bank
        ):
            # ---------------- constants / inputs -----------------------
            xt = cpool.tile([P, ET, S], dt.bfloat16)
            wq = cpool.tile([P, ET, 512], dt.bfloat16)
            wk = cpool.tile([P, ET, 512], dt.bfloat16)
            wv = cpool.tile([P, ET, 512], dt.bfloat16)
            wo = cpool.tile([P, DT, E], dt.bfloat16)
            bq = cpool.tile([P, DT], dt.float32)
            bk = cpool.tile([P, DT], dt.float32)
            bv = cpool.tile([1, 512], dt.bfloat16)
            bo = cpool.tile([P, ET], dt.float32)
            iden = cpool.tile([P, P], dt.bfloat16)
            ones1 = cpool.tile([1, P], dt.bfloat16)

            # activations
            qt8 = apool.tile([P, DT, S], dt.float8e4)   # Q^T (bias+fp8)
            kt8 = apool.tile([P, DT, S], dt.float8e4)   # K^T (bias+fp8)
            va = apool.tile([P, ST, NH, HD + 1], dt.bfloat16)  # V | ones
            scb = apool.tile([P, DT, S], dt.bfloat16)   # O^T (normalized)

            # long-lived PSUM tiles (sub-region dependency tracking)
            ring = scpool.tile([P, 4, 512], dt.float32, tag="sc", name="ring")
            pv = pvpool.tile([P, 8, P], dt.float32, tag="pv", name="pv")

            # DMA transfers serialize on the global DMA device, so order by
            # criticality: d-tile-0 K/Q slices and head-0 V cols first, the
            # full x^T (bandwidth floor ~12.6us), then everything else.
            wkr = wk_d.rearrange("(eo p) c -> p eo c", p=P)
            wqr = wq_d.rearrange("(eo p) c -> p eo c", p=P)
            wvr = wv_d.rearrange("(eo p) c -> p eo c", p=P)
            nc.sync.dma_start(wk[:, :, 0:P], wkr[:, :, 0:P])
            nc.sync.dma_start(bk[:], bk_d)
            for e in range(ET):
                nc.sync.dma_start(xt[:, e, :], xt_d[e * P : (e + 1) * P, :])
            nc.sync.dma_start(wq[:, :, 0:P], wqr[:, :, 0:P])
            nc.sync.dma_start(bq[:], bq_d)
            nc.sync.dma_start(wv[:, :, 0:HD], wvr[:, :, 0:HD])
            nc.sync.dma_start(bv[:], bv_d)
            nc.sync.dma_start(wk[:, :, P:512], wkr[:, :, P:512])
            nc.sync.dma_start(wq[:, :, P:512], wqr[:, :, P:512])
            nc.sync.dma_start(wv[:, :, HD:512], wvr[:, :, HD:512])
            nc.sync.dma_start(iden[:], iden_d)
            nc.sync.dma_start(wo[:], wo_d.rearrange("(eo p) c -> p eo c", p=P))
            nc.sync.dma_start(bo[:], bo_d)
            nc.gpsimd.memset(ones1[:], 1.0)
            nc.gpsimd.memset(va[:, :, :, HD : HD + 1], 1.0)

            # ---------------- small-chunk emitters ----------------------
            # kq/outproj chunks come as (partA, partB) sharing one psum tile;
            # with a single-buffer proj pool they are also adjacent-safe.

            def kq_halves(which, t, c):
                w_, b_, dst = (wk, bk, kt8) if which == "k" else (wq, bq, qt8)
                holder = {}

                def part0():
                    ps = pjpool.tile(
                        [P, 512], dt.float32, tag="pj", name=f"{which}{t}{c}"
                    )
                    holder[0] = ps
                    for e in range(4):
                        nc.tensor.matmul(
                            ps[:],
                            w_[:, e, t * P : (t + 1) * P],
                            xt[:, e, c * 512 : (c + 1) * 512],
                            start=(e == 0),
                            stop=False,
                        )

                def part1():
                    ps = holder[0]
                    for e in range(4, ET):
                        nc.tensor.matmul(
                            ps[:],
                            w_[:, e, t * P : (t + 1) * P],
                            xt[:, e, c * 512 : (c + 1) * 512],
                            start=False,
                            stop=(e == ET - 1),
                        )
                    nc.vector.tensor_scalar_add(
                        dst[:, t, c * 512 : (c + 1) * 512], ps[:], b_[:, t : t + 1]
                    )

                return [part0, part1]

            def v_chunk(h, st):
                """V rows for (head h, key tile st): [128 keys, 64] + bias.
                Uses the transpose-psum bank (tiny tiles) to stay off the
                kq/outproj pipeline."""
                def go():
                    ps = tppool.tile([P, P], dt.float32, tag="vps", name=f"v{h}{st}")
                    for e in range(ET):
                        nc.tensor.matmul(
                            ps[:, 0:HD],
                            xt[:, e, st * P : (st + 1) * P],
                            wv[:, e, h * HD : (h + 1) * HD],
                            start=(e == 0),
                            stop=False,
                        )
                    nc.tensor.matmul(
                        ps[:, 0:HD],
                        ones1[0:1, :],
                        bv[0:1, h * HD : (h + 1) * HD],
                        start=False,
                        stop=True,
                    )
                    nc.vector.tensor_copy(va[:, st, h, 0:HD], ps[:, 0:HD])
                return go

            def outproj_halves(eo, qq):
                """Partial out^T tile [128 Eo, 512 q] for query quarter qq."""
                holder = {}
                q0 = qq * 512

                def part0():
                    ps = pjpool.tile([P, 512], dt.float32, tag="pj", name=f"o{eo}{qq}")
                    holder[0] = ps
                    for t in (0, 1):
                        nc.tensor.matmul(
                            ps[:],
                            wo[:, t, eo * P : (eo + 1) * P],
                            scb[:, t, q0 : q0 + 512],
                            start=(t == 0),
                            stop=False,
                        )

                def part1():
                    ps = holder[0]
                    for t in (2, 3):
                        nc.tensor.matmul(
                            ps[:],
                            wo[:, t, eo * P : (eo + 1) * P],
                            scb[:, t, q0 : q0 + 512],
                            start=False,
                            stop=(t == DT - 1),
                        )
                    ot = outpool.tile(
                        [P, 512], dt.float32, tag="ot", name=f"oe{eo}{qq}"
                    )
                    nc.vector.tensor_scalar_add(ot[:], ps[:], bo[:, eo : eo + 1])
                    nc.sync.dma_start(
                        out_d[eo * P : (eo + 1) * P, q0 : q0 + 512], ot[:]
                    )

                return [part0, part1]

            oq_tiles = {}

            def transpose_item(t, qh, qt):
                """oq [q, dd of pair t] -> scb[:, t, ...] via PE transpose.
                Uses the same psum bank as v_chunk (different tag would
                double-book the bank, so share tag/shape via bitcast)."""
                def go():
                    tp = tppool.tile([P, P], dt.float32, tag="vps", name=f"tp{t}{qh}{qt}")
                    tpb = tp[:, 0:HD].bitcast(dt.bfloat16)
                    nc.tensor.transpose(tpb, oq_tiles[(t, qh)][:, qt, :], iden[:])
                    q0 = qh * QH + qt * P
                    nc.vector.tensor_copy(scb[:, t, q0 : q0 + P], tpb)
                return go

            # ---------------- attention stream ---------------------------
            # One global stream of 256 j-steps (16 phases x 16 key tiles).
            # Per step: exp(step) on ACT, then on PE scores(step+2) (its ring
            # slots were just freed by exp(step)), then the PV wave of step,
            # then interleaved thunks.  This keeps the serial chain between
            # consecutive exps down to one sem hop, across phase boundaries
            # included.
            def scores(step):
                h, qh, j = phase_of(step)
                t, hp = h // 2, (h % 2) * HD
                q0 = qh * QH
                s = (2 * step) % 4
                for qc in range(2):
                    nc.tensor.matmul(
                        ring[:, s + qc, :],
                        kt8[hp : hp + HD, t, j * P : (j + 1) * P]
                        .unsqueeze(1)
                        .broadcast_to((HD, 2, P)),
                        qt8[hp : hp + HD, t, q0 + qc * 512 : q0 + (qc + 1) * 512]
                        .unsqueeze(1)
                        .broadcast_to((HD, 2, 512)),
                        start=True,
                        stop=True,
                        perf_mode=DR,
                    )

            def phase_of(step):
                phase, j = divmod(step, ST)
                qh, h = divmod(phase, NH)
                return h, qh, j

            def emit_evac(h, qh):
                t, half = h // 2, h % 2
                if half == 0:
                    oq_tiles[(t, qh)] = oqpool.tile(
                        [P, 8, P], dt.bfloat16, tag="oq", name=f"oq{t}{qh}"
                    )
                oq = oq_tiles[(t, qh)]
                rec = recpool.tile([P, 8], dt.float32, tag="rec", name=f"rc{h}{qh}")
                scr = recpool.tile([P, 8], dt.float32, tag="scr", name=f"sr{h}{qh}")
                nc.vector.reciprocal_approx_accurate(
                    rec[:], pv[:, :, HD : HD + 1].rearrange("p a b -> p (a b)"), scr[:]
                )
                for qt in range(8):
                    nc.vector.tensor_scalar(
                        oq[:, qt, half * HD : (half + 1) * HD],
                        pv[:, qt, 0:HD],
                        rec[:, qt : qt + 1],
                        None,
                        op0=mybir.AluOpType.mult,
                    )

            def run_stream(work):
                """work: list of dicts {release, due, fns: [(fn, cost), ...]}.
                Per step, spend ~STEP_BUDGET ns of PE time on the earliest-due
                released items; a partially-emitted item always continues
                first (its parts share one psum tile)."""
                STEP_BUDGET = 620.0
                n_steps = 16 * NH * 2
                scores(0)
                scores(1)
                pending = sorted(work, key=lambda w: (w["due"], w["release"]))
                current = None
                for step in range(n_steps):
                    h, qh, j = phase_of(step)
                    ptile = ppool.tile(
                        [P, 2, 512], dt.bfloat16, tag="p", name=f"p{step}"
                    )
                    s = (2 * step) % 4
                    nc.scalar.activation(
                        ptile.rearrange("p a b -> p (a b)"),
                        ring[:, s : s + 2, :].rearrange("p a b -> p (a b)"),
                        mybir.ActivationFunctionType.Exp,
                        scale=SCALE / 2.0,
                    )
                    if step + 2 < n_steps:
                        scores(step + 2)
                    for qt in range(8):
                        nc.tensor.matmul(
                            pv[:, qt, 0 : HD + 1],
                            ptile[:, qt // 4, (qt % 4) * P : (qt % 4 + 1) * P],
                            va[:, j, h, :],
                            start=(j == 0 and qt % 4 == 0),
                            stop=(j == ST - 1),
                            skip_group_check=True,
                        )
                    budget = STEP_BUDGET
                    while budget > 0:
                        if current is None:
                            cand = [w for w in pending if w["release"] <= step]
                            if not cand:
                                break
                            current = cand[0]
                            pending.remove(current)
                            assert current["due"] >= step, (
                                f"work item overdue: emitted step {step}, "
                                f"due {current['due']}"
                            )
                        fn, cost = current["fns"].pop(0)
                        fn()
                        budget -= cost
                        if not current["fns"]:
                            current = None
                    if j == ST - 1:
                        emit_evac(h, qh)
                # anything left (tail work: release >= n_steps)
                leftovers = ([current] if current else []) + pending
                leftovers.sort(key=lambda w: (w["release"], w["due"]))
                for w in leftovers:
                    for fn, _ in w["fns"]:
                        fn()

            # ---------------- emission schedule -------------------------
            # Pre-attention prefix: K/Q d-tile 0 for the first half, with
            # V chunks (separate psum bank) filling the proj-evac gaps.
            prefix_kq = (
                kq_halves("k", 0, 0) + kq_halves("q", 0, 0)
                + kq_halves("k", 0, 1) + kq_halves("q", 0, 1)
            )
            prefix_v = [v_chunk(0, st) for st in range(6)]
            for i, fn in enumerate(prefix_kq):
                fn()
                if i % 2 == 1 and prefix_v:
                    prefix_v.pop(0)()
            for fn in prefix_v:
                fn()

            # Work items with release/due steps.  Due dates: a K/Q chunk
            # feeding scores(x) must finish by step x-3 (scores run two
            # steps ahead and precede thunks within a step); a V chunk
            # feeding PV(x) by step x-1.
            KQC, VC, TRC, OPC = 950.0, 340.0, 250.0, 650.0
            work = []

            def add(release, due, fns, cost):
                work.append(
                    {"release": release, "due": due,
                     "fns": [(f, cost) for f in fns]}
                )

            # V: head 0 tiles 6..15 (0..5 in the prefix), then all other heads
            for st in range(6, ST):
                add(0, st - 1, [v_chunk(0, st)], VC)
            for h in range(1, NH):
                for st in range(ST):
                    add(0, 16 * h + st - 1, [v_chunk(h, st)], VC)
            # K: d-tile 0 chunks 2/3 (0/1 in the prefix), d-tiles 1..3 all
            for c in (2, 3):
                add(0, 4 * c - 3, kq_halves("k", 0, c), KQC / 2)
            for t2 in range(1, DT):
                for c in range(4):
                    add(0, 32 * t2 + 4 * c - 3, kq_halves("k", t2, c), KQC / 2)
            # Q: low half (chunks 0/1) due at (2t, qh0); high half at qh1
            for t2 in range(1, DT):
                for c in (0, 1):
                    add(0, 32 * t2 - 3, kq_halves("q", t2, c), KQC / 2)
            for t2 in range(DT):
                for c in (2, 3):
                    add(0, 128 + 32 * t2 - 3, kq_halves("q", t2, c), KQC / 2)
            # transposes: half-0 pairs due before outproj(qq 0/1) releases;
            # half-1 pairs before the tail outproj
            for t2 in range(DT):
                add(32 * t2 + 32, 150, [transpose_item(t2, 0, qt) for qt in range(8)], TRC)
            for t2 in range(DT):
                add(176 + 32 * t2, 255 if t2 < DT - 1 else 10**6,
                    [transpose_item(t2, 1, qt) for qt in range(8)], TRC)
            # outproj quarters 0/1 during half 1; quarters 2/3 in the tail
            for eo in range(ET):
                for qq in (0, 1):
                    add(140, 254, outproj_halves(eo, qq), OPC)
            # tail outproj uses the (then idle) scores ring as rotating psum
            ci = [0]

            def tail_outproj(eo, qq):
                def go():
                    ps = ring[:, ci[0] % 4, :]
                    ci[0] += 1
                    q0 = qq * 512
                    for t in range(DT):
                        nc.tensor.matmul(
                            ps,
                            wo[:, t, eo * P : (eo + 1) * P],
                            scb[:, t, q0 : q0 + 512],
                            start=(t == 0),
                            stop=(t == DT - 1),
                        )
                    ot = outpool.tile(
                        [P, 512], dt.float32, tag="ot", name=f"ot{eo}{qq}"
                    )
                    nc.vector.tensor_scalar_add(ot[:], ps, bo[:, eo : eo + 1])
                    nc.sync.dma_start(
                        out_d[eo * P : (eo + 1) * P, q0 : q0 + 512], ot[:]
                    )
                return go

            for eo in range(ET):
                for qq in (2, 3):
                    add(10**6, 10**6, [tail_outproj(eo, qq)], OPC)

            # Run the whole attention stream (tail work included).
            run_stream(work)

    nc.compile()
    return nc


def _prep_inputs(x, W_qkv, b_qkv, W_out, b_out):
    """Host-side sharding + layout prep. Returns per-core input maps."""
    w = W_qkv.reshape(E, H, 3, HD)
    b3 = b_qkv.reshape(H, 3, HD)
    iden = np.eye(P, dtype=np.float32).astype(_BF16)

    in_maps = []
    for core in range(N_CORES):
        b, hg = core // 2, core % 2
        hs = slice(hg * NH, (hg + 1) * NH)
        xt = np.ascontiguousarray(x[b].T).astype(_BF16)           # [E, S]
        wq = np.ascontiguousarray(w[:, hs, 0, :].reshape(E, 512)).astype(_BF16)
        wk = np.ascontiguousarray(w[:, hs, 1, :].reshape(E, 512)).astype(_BF16)
        wv = np.ascontiguousarray(w[:, hs, 2, :].reshape(E, 512)).astype(_BF16)
        wo = np.ascontiguousarray(W_out[hg * 512 : (hg + 1) * 512, :]).astype(_BF16)
        bq = np.ascontiguousarray(b3[hs, 0, :].reshape(DT, P).T).astype(np.float32)
        bk = np.ascontiguousarray(b3[hs, 1, :].reshape(DT, P).T).astype(np.float32)
        bv = np.ascontiguousarray(b3[hs, 2, :].reshape(1, 512)).astype(_BF16)
        bo = (np.ascontiguousarray(b_out.reshape(ET, P).T) * (1.0 if hg == 0 else 0.0)).astype(np.float32)
        in_maps.append(
            {
                "xt": xt,
                "wq": wq,
                "wk": wk,
                "wv": wv,
                "wo": wo,
                "bq": bq,
                "bk": bk,
                "bv": bv,
                "bo": bo,
                "iden": iden,
            }
        )
    return in_maps


def run_raw(x, W_qkv, b_qkv, W_out, b_out, trace=False, **kw):
    """Run on hardware; returns (full_output [B,S,E] f32, BassKernelResults)."""
    global _cached
    from concourse.bass_utils import run_bass_kernel_spmd

    if _cached is None:
        _cached = _build()
    nc = _cached
    in_maps = _prep_inputs(
        np.asarray(x), np.asarray(W_qkv), np.asarray(b_qkv),
        np.asarray(W_out), np.asarray(b_out),
    )
    res = run_bass_kernel_spmd(
        nc, in_maps, core_ids=list(range(N_CORES)), trace=trace, **kw
    )
    out = np.empty((B, S, E), dtype=np.float32)
    for b in range(B):
        acc = np.asarray(res.results[2 * b]["out"]) + np.asarray(
            res.results[2 * b + 1]["out"]
        )
        out[b] = acc.T
    return out, res


def kernel(x, W_qkv, b_qkv, W_out, b_out):
    out, _ = run_raw(x, W_qkv, b_qkv, W_out, b_out, trace=False)
    return out


# revision 32
# speedup vs baseline: 1.3858x; 1.2496x over previous
"""Multi-head attention (B=4, S=2048, E=1024, H=16) on 8 TRN2 NeuronCores.

Sharding: batch x head-group tensor parallel -- core c = 2*b + hg handles
batch b and heads hg*8 .. hg*8+7 for ALL 2048 queries.  Q/K/V projections
are column-split by head (each core projects only its 8 heads); the output
projection is row-split (each core contracts its 512 E-rows of W_out) and
produces a partial [E, S] output that the HOST sums across the core pair
while unsharding (the "all-reduce" of the sharding hint, done on host).

Per-core kernel:
  - Q^T/K^T projections (bf16 matmul, fp32 PSUM) evacuated with fused
    bias-add + fp8e4 quantization (DVE tensor_scalar_add, fp8 out).
  - scores via fp8 DoubleRow matmuls: contraction d=64 fed as
    [64 part, 2(dup, stride 0), N]; the duplicated group doubles the
    result and the exp activation scale absorbs the factor 2.
    Cost: 0.5 cycles/row (vs 1.0 bf16).
  - exp on ScalarE (the bottleneck engine, ~266us busy): one [128, 1024]
    activation per key-tile j, reading two adjacent 512-wide slots of a
    manual 4-slot PSUM ring (slots 2j%4, 2j%4+1 -> flat AP; the two slot
    pairs double-buffer).  Scores for j+1 are emitted BEFORE the PV
    matmuls of j so the exp stream never waits on PE's in-order queue.
  - PV in the FLIPPED orientation: out[q=128, 65] = P_tile.T @ [V | ones]
    (all 128 output partitions vs 65 the naive way); the ones column is
    the softmax denominator per query row.  The 8 per-qt accumulators
    live in one [128, 8, 128] tile (qt stride 512B -> no bank crossing);
    PSUM start=True zero-fills a whole 2KB region, so only the first
    matmul touching each bank uses start=True and the rest rely on the
    pending-zero overwrite semantics (no memset needed).
  - normalization: per-partition reciprocal of the denominator column +
    tensor_scalar multiply -> O in [q, d]; PE-transpose (identity
    matmul) back to O^T for the out projection.

Schedule: 2 query-half phase groups x 8 heads x 16 key tiles (j).
Per j-step the PE also runs one or two small interleaved chunks: V
projection for the NEXT head (just-in-time, ~240ns each), K/Q
projection half-chunks (~850ns), O^T transposes, and the first half's
output projection (during the second half).  Only the second half's
output projection is a serial tail.
"""

import sys

if "/opt/trn_rl_repo" not in sys.path:
    sys.path.insert(0, "/opt/trn_rl_repo")

import numpy as np
import ml_dtypes

B, S, E, H = 4, 2048, 1024, 16
P = 128
HD = 64           # head dim
NH = 8            # heads per core
DT = 4            # d-tiles (head pairs) per core
ET = E // P       # 8 e-tiles (contraction for projections)
ST = S // P       # 16 key tiles
N_CORES = 8
QH = S // 2       # query half width (1024)
SCALE = 1.0 / float(np.sqrt(HD))

_BF16 = ml_dtypes.bfloat16

_cached = None


def _build():
    import concourse.bass as bass
    import concourse.tile as tile
    import concourse.mybir as mybir
    from concourse import bacc

    dt = mybir.dt
    nc = bacc.Bacc("TRN2", target_bir_lowering=False, debug=False)

    xt_d = nc.dram_tensor("xt", [E, S], dt.bfloat16, kind="ExternalInput").ap()
    wq_d = nc.dram_tensor("wq", [E, 512], dt.bfloat16, kind="ExternalInput").ap()
    wk_d = nc.dram_tensor("wk", [E, 512], dt.bfloat16, kind="ExternalInput").ap()
    wv_d = nc.dram_tensor("wv", [E, 512], dt.bfloat16, kind="ExternalInput").ap()
    wo_d = nc.dram_tensor("wo", [512, E], dt.bfloat16, kind="ExternalInput").ap()
    bq_d = nc.dram_tensor("bq", [P, DT], dt.float32, kind="ExternalInput").ap()
    bk_d = nc.dram_tensor("bk", [P, DT], dt.float32, kind="ExternalInput").ap()
    bv_d = nc.dram_tensor("bv", [1, 512], dt.bfloat16, kind="ExternalInput").ap()
    bo_d = nc.dram_tensor("bo", [P, ET], dt.float32, kind="ExternalInput").ap()
    iden_d = nc.dram_tensor("iden", [P, P], dt.bfloat16, kind="ExternalInput").ap()
    out_d = nc.dram_tensor("out", [E, S], dt.float32, kind="ExternalOutput").ap()

    DR = mybir.MatmulPerfMode.DoubleRow

    with tile.TileContext(nc) as tc:
        with (
            tc.tile_pool(name="const", bufs=1) as cpool,
            tc.tile_pool(name="acts", bufs=1) as apool,
            tc.tile_pool(name="pp", bufs=3) as ppool,        # P (exp out)
            tc.tile_pool(name="oqp", bufs=2) as oqpool,      # O [q, dd] staging
            tc.tile_pool(name="recp", bufs=2) as recpool,    # reciprocals
            tc.tile_pool(name="outs", bufs=8) as outpool,    # out staging
            tc.tile_pool(name="pssc", bufs=2, space="PSUM") as scpool,   # 4 banks
            tc.tile_pool(name="pspv", bufs=1, space="PSUM") as pvpool,   # 2 banks
            tc.tile_pool(name="pspj", bufs=1, space="PSUM") as pjpool,   # 1 bank
            tc.tile_pool(name="pstp", bufs=1, space="PSUM") as tppool,   # 1 bank
        ):
            # ---------------- constants / inputs -----------------------
            xt = cpool.tile([P, ET, S], dt.bfloat16)
            wq = cpool.tile([P, ET, 512], dt.bfloat16)
            wk = cpool.tile([P, ET, 512], dt.bfloat16)
            wv = cpool.tile([P, ET, 512], dt.bfloat16)
            wo = cpool.tile([P, DT, E], dt.bfloat16)
            bq = cpool.tile([P, DT], dt.float32)
            bk = cpool.tile([P, DT], dt.float32)
            bv = cpool.tile([1, 512], dt.bfloat16)
            bo = cpool.tile([P, ET], dt.float32)
            iden = cpool.tile([P, P], dt.bfloat16)
            ones1 = cpool.tile([1, P], dt.bfloat16)

            # activations
            qt8 = apool.tile([P, DT, S], dt.float8e4)   # Q^T (bias+fp8)
            kt8 = apool.tile([P, DT, S], dt.float8e4)   # K^T (bias+fp8)
            va = apool.tile([P, ST, NH, HD + 1], dt.bfloat16)  # V | ones
            scb = apool.tile([P, DT, S], dt.bfloat16)   # O^T (normalized)

            # Long-lived PSUM tiles.  Dependency tracking is TILE-granular,
            # so the scores ring is TWO alternating tiles: exp(step) then
            # only depends on its own tile's scores, and scores(step+2)
            # (same tile) WAR-waits exp(step) -- the other tile streams
            # freely underneath.
            rings = [
                scpool.tile([P, 2, 512], dt.float32, tag="sc", name="ringA"),
                scpool.tile([P, 2, 512], dt.float32, tag="sc", name="ringB"),
            ]
            pv = pvpool.tile([P, 8, P], dt.float32, tag="pv", name="pv")

            # DMA transfers serialize on the global DMA device, so order by
            # criticality: d-tile-0 K/Q slices and head-0 V cols first, the
            # full x^T (bandwidth floor ~12.6us), then everything else.
            wkr = wk_d.rearrange("(eo p) c -> p eo c", p=P)
            wqr = wq_d.rearrange("(eo p) c -> p eo c", p=P)
            wvr = wv_d.rearrange("(eo p) c -> p eo c", p=P)
            nc.sync.dma_start(wk[:, :, 0:P], wkr[:, :, 0:P])
            nc.sync.dma_start(bk[:], bk_d)
            for e in range(ET):
                nc.sync.dma_start(xt[:, e, :], xt_d[e * P : (e + 1) * P, :])
            nc.sync.dma_start(wq[:, :, 0:P], wqr[:, :, 0:P])
            nc.sync.dma_start(bq[:], bq_d)
            nc.sync.dma_start(wv[:, :, 0:HD], wvr[:, :, 0:HD])
            nc.sync.dma_start(bv[:], bv_d)
            nc.sync.dma_start(wk[:, :, P:512], wkr[:, :, P:512])
            nc.sync.dma_start(wq[:, :, P:512], wqr[:, :, P:512])
            nc.sync.dma_start(wv[:, :, HD:512], wvr[:, :, HD:512])
            nc.sync.dma_start(iden[:], iden_d)
            nc.sync.dma_start(wo[:], wo_d.rearrange("(eo p) c -> p eo c", p=P))
            nc.sync.dma_start(bo[:], bo_d)
            nc.gpsimd.memset(ones1[:], 1.0)
            nc.gpsimd.memset(va[:, :, :, HD : HD + 1], 1.0)

            # ---------------- small-chunk emitters ----------------------
            # kq/outproj chunks come as (partA, partB) sharing one psum tile;
            # with a single-buffer proj pool they are also adjacent-safe.

            def kq_halves(which, t, c):
                w_, b_, dst = (wk, bk, kt8) if which == "k" else (wq, bq, qt8)
                holder = {}

                def part0():
                    ps = pjpool.tile(
                        [P, 512], dt.float32, tag="pj", name=f"{which}{t}{c}"
                    )
                    holder[0] = ps
                    for e in range(4):
                        nc.tensor.matmul(
                            ps[:],
                            w_[:, e, t * P : (t + 1) * P],
                            xt[:, e, c * 512 : (c + 1) * 512],
                            start=(e == 0),
                            stop=False,
                        )

                def part1():
                    ps = holder[0]
                    for e in range(4, ET):
                        nc.tensor.matmul(
                            ps[:],
                            w_[:, e, t * P : (t + 1) * P],
                            xt[:, e, c * 512 : (c + 1) * 512],
                            start=False,
                            stop=(e == ET - 1),
                        )
                    nc.vector.tensor_scalar_add(
                        dst[:, t, c * 512 : (c + 1) * 512], ps[:], b_[:, t : t + 1]
                    )

                return [part0, part1]

            def v_chunk(h, st):
                """V rows for (head h, key tile st): [128 keys, 64] + bias.
                Uses the transpose-psum bank (tiny tiles) to stay off the
                kq/outproj pipeline."""
                def go():
                    ps = tppool.tile([P, P], dt.float32, tag="vps", name=f"v{h}{st}")
                    for e in range(ET):
                        nc.tensor.matmul(
                            ps[:, 0:HD],
                            xt[:, e, st * P : (st + 1) * P],
                            wv[:, e, h * HD : (h + 1) * HD],
                            start=(e == 0),
                            stop=False,
                        )
                    nc.tensor.matmul(
                        ps[:, 0:HD],
                        ones1[0:1, :],
                        bv[0:1, h * HD : (h + 1) * HD],
                        start=False,
                        stop=True,
                    )
                    nc.vector.tensor_copy(va[:, st, h, 0:HD], ps[:, 0:HD])
                return go

            def outproj_halves(eo, qq):
                """Partial out^T tile [128 Eo, 512 q] for query quarter qq."""
                holder = {}
                q0 = qq * 512

                def part0():
                    ps = pjpool.tile([P, 512], dt.float32, tag="pj", name=f"o{eo}{qq}")
                    holder[0] = ps
                    for t in (0, 1):
                        nc.tensor.matmul(
                            ps[:],
                            wo[:, t, eo * P : (eo + 1) * P],
                            scb[:, t, q0 : q0 + 512],
                            start=(t == 0),
                            stop=False,
                        )

                def part1():
                    ps = holder[0]
                    for t in (2, 3):
                        nc.tensor.matmul(
                            ps[:],
                            wo[:, t, eo * P : (eo + 1) * P],
                            scb[:, t, q0 : q0 + 512],
                            start=False,
                            stop=(t == DT - 1),
                        )
                    ot = outpool.tile(
                        [P, 512], dt.float32, tag="ot", name=f"oe{eo}{qq}"
                    )
                    nc.vector.tensor_scalar_add(ot[:], ps[:], bo[:, eo : eo + 1])
                    nc.sync.dma_start(
                        out_d[eo * P : (eo + 1) * P, q0 : q0 + 512], ot[:]
                    )

                return [part0, part1]

            oq_tiles = {}

            def transpose_item(t, qh, qt):
                """oq [q, dd of pair t] -> scb[:, t, ...] via PE transpose.
                Uses the same psum bank as v_chunk (different tag would
                double-book the bank, so share tag/shape via bitcast)."""
                def go():
                    tp = tppool.tile([P, P], dt.float32, tag="vps", name=f"tp{t}{qh}{qt}")
                    tpb = tp[:, 0:HD].bitcast(dt.bfloat16)
                    nc.tensor.transpose(tpb, oq_tiles[(t, qh)][:, qt, :], iden[:])
                    q0 = qh * QH + qt * P
                    nc.vector.tensor_copy(scb[:, t, q0 : q0 + P], tpb)
                return go

            # ---------------- attention stream ---------------------------
            # One global stream of 256 j-steps (16 phases x 16 key tiles).
            # Per step: exp(step) on ACT, then on PE scores(step+2) (its ring
            # slots were just freed by exp(step)), then the PV wave of step,
            # then interleaved thunks.  This keeps the serial chain between
            # consecutive exps down to one sem hop, across phase boundaries
            # included.
            def scores(step):
                h, qh, j = phase_of(step)
                t, hp = h // 2, (h % 2) * HD
                q0 = qh * QH
                rg = rings[step % 2]
                for qc in range(2):
                    nc.tensor.matmul(
                        rg[:, qc, :],
                        kt8[hp : hp + HD, t, j * P : (j + 1) * P]
                        .unsqueeze(1)
                        .broadcast_to((HD, 2, P)),
                        qt8[hp : hp + HD, t, q0 + qc * 512 : q0 + (qc + 1) * 512]
                        .unsqueeze(1)
                        .broadcast_to((HD, 2, 512)),
                        start=True,
                        stop=True,
                        perf_mode=DR,
                    )

            def phase_of(step):
                phase, j = divmod(step, ST)
                qh, h = divmod(phase, NH)
                return h, qh, j

            def emit_evac(h, qh):
                t, half = h // 2, h % 2
                if half == 0:
                    oq_tiles[(t, qh)] = oqpool.tile(
                        [P, 8, P], dt.bfloat16, tag="oq", name=f"oq{t}{qh}"
                    )
                oq = oq_tiles[(t, qh)]
                rec = recpool.tile([P, 8], dt.float32, tag="rec", name=f"rc{h}{qh}")
                scr = recpool.tile([P, 8], dt.float32, tag="scr", name=f"sr{h}{qh}")
                nc.vector.reciprocal_approx_accurate(
                    rec[:], pv[:, :, HD : HD + 1].rearrange("p a b -> p (a b)"), scr[:]
                )
                for qt in range(8):
                    nc.vector.tensor_scalar(
                        oq[:, qt, half * HD : (half + 1) * HD],
                        pv[:, qt, 0:HD],
                        rec[:, qt : qt + 1],
                        None,
                        op0=mybir.AluOpType.mult,
                    )

            def run_stream(work):
                """work: list of dicts {release, due, fns: [(fn, cost), ...]}.
                Per step, spend ~STEP_BUDGET ns of PE time on the earliest-due
                released items; a partially-emitted item always continues
                first (its parts share one psum tile)."""
                STEP_BUDGET = 620.0
                n_steps = 16 * NH * 2
                scores(0)
                scores(1)
                pending = sorted(work, key=lambda w: (w["due"], w["release"]))
                current = None
                for step in range(n_steps):
                    h, qh, j = phase_of(step)
                    ptile = ppool.tile(
                        [P, 2, 512], dt.bfloat16, tag="p", name=f"p{step}"
                    )
                    nc.scalar.activation(
                        ptile.rearrange("p a b -> p (a b)"),
                        rings[step % 2].rearrange("p a b -> p (a b)"),
                        mybir.ActivationFunctionType.Exp,
                        scale=SCALE / 2.0,
                    )
                    if step + 2 < n_steps:
                        scores(step + 2)
                    for qt in range(8):
                        nc.tensor.matmul(
                            pv[:, qt, 0 : HD + 1],
                            ptile[:, qt // 4, (qt % 4) * P : (qt % 4 + 1) * P],
                            va[:, j, h, :],
                            start=(j == 0 and qt % 4 == 0),
                            stop=(j == ST - 1),
                            skip_group_check=True,
                        )
                    budget = STEP_BUDGET
                    while budget > 0:
                        if current is None:
                            cand = [w for w in pending if w["release"] <= step]
                            if not cand:
                                break
                            current = cand[0]
                            pending.remove(current)
                            assert current["due"] >= step, (
                                f"work item overdue: emitted step {step}, "
                                f"due {current['due']}"
                            )
                        fn, cost = current["fns"].pop(0)
                        fn()
                        budget -= cost
                        if not current["fns"]:
                            current = None
                    if j == ST - 1:
                        emit_evac(h, qh)
                # anything left (tail work: release >= n_steps)
                leftovers = ([current] if current else []) + pending
                leftovers.sort(key=lambda w: (w["release"], w["due"]))
                for w in leftovers:
                    for fn, _ in w["fns"]:
                        fn()

            # ---------------- emission schedule -------------------------
            # Pre-attention prefix: K/Q d-tile 0 for the first half, with
            # V chunks (separate psum bank) filling the proj-evac gaps.
            prefix_kq = (
                kq_halves("k", 0, 0) + kq_halves("q", 0, 0)
                + kq_halves("k", 0, 1) + kq_halves("q", 0, 1)
            )
            prefix_v = [v_chunk(0, st) for st in range(6)]
            for i, fn in enumerate(prefix_kq):
                fn()
                if i % 2 == 1 and prefix_v:
                    prefix_v.pop(0)()
            for fn in prefix_v:
                fn()

            # Work items with release/due steps.  Due dates: a K/Q chunk
            # feeding scores(x) must finish by step x-3 (scores run two
            # steps ahead and precede thunks within a step); a V chunk
            # feeding PV(x) by step x-1.
            KQC, VC, TRC, OPC = 950.0, 340.0, 250.0, 650.0
            work = []

            def add(release, due, fns, cost):
                work.append(
                    {"release": release, "due": due,
                     "fns": [(f, cost) for f in fns]}
                )

            # V: head 0 tiles 6..15 (0..5 in the prefix), then all other heads
            for st in range(6, ST):
                add(0, st - 1, [v_chunk(0, st)], VC)
            for h in range(1, NH):
                for st in range(ST):
                    add(0, 16 * h + st - 1, [v_chunk(h, st)], VC)
            # K: d-tile 0 chunks 2/3 (0/1 in the prefix), d-tiles 1..3 all
            for c in (2, 3):
                add(0, 4 * c - 3, kq_halves("k", 0, c), KQC / 2)
            for t2 in range(1, DT):
                for c in range(4):
                    add(0, 32 * t2 + 4 * c - 3, kq_halves("k", t2, c), KQC / 2)
            # Q: low half (chunks 0/1) due at (2t, qh0); high half at qh1
            for t2 in range(1, DT):
                for c in (0, 1):
                    add(0, 32 * t2 - 3, kq_halves("q", t2, c), KQC / 2)
            for t2 in range(DT):
                for c in (2, 3):
                    add(0, 128 + 32 * t2 - 3, kq_halves("q", t2, c), KQC / 2)
            # transposes: half-0 pairs due before outproj(qq 0/1) releases;
            # half-1 pairs before the tail outproj
            for t2 in range(DT):
                add(32 * t2 + 32, 150, [transpose_item(t2, 0, qt) for qt in range(8)], TRC)
            for t2 in range(DT):
                add(176 + 32 * t2, 255 if t2 < DT - 1 else 10**6,
                    [transpose_item(t2, 1, qt) for qt in range(8)], TRC)
            # outproj quarters 0/1 during half 1; quarters 2/3 in the tail
            for eo in range(ET):
                for qq in (0, 1):
                    add(140, 254, outproj_halves(eo, qq), OPC)
            # tail outproj uses the (then idle) scores ring as rotating psum
            ci = [0]

            def tail_outproj(eo, qq):
                def go():
                    ps = rings[ci[0] % 2][:, (ci[0] // 2) % 2, :]
                    ci[0] += 1
                    q0 = qq * 512
                    for t in range(DT):
                        nc.tensor.matmul(
                            ps,
                            wo[:, t, eo * P : (eo + 1) * P],
                            scb[:, t, q0 : q0 + 512],
                            start=(t == 0),
                            stop=(t == DT - 1),
                        )
                    ot = outpool.tile(
                        [P, 512], dt.float32, tag="ot", name=f"ot{eo}{qq}"
                    )
                    nc.vector.tensor_scalar_add(ot[:], ps, bo[:, eo : eo + 1])
                    nc.sync.dma_start(
                        out_d[eo * P : (eo + 1) * P, q0 : q0 + 512], ot[:]
                    )
                return go

            for eo in range(ET):
                for qq in (2, 3):
                    add(10**6, 10**6, [tail_outproj(eo, qq)], OPC)

            # Run the whole attention stream (tail work included).
            run_stream(work)

    nc.compile()
    return nc


def _prep_inputs(x, W_qkv, b_qkv, W_out, b_out):
    """Host-side sharding + layout prep. Returns per-core input maps."""
    w = W_qkv.reshape(E, H, 3, HD)
    b3 = b_qkv.reshape(H, 3, HD)
    iden = np.eye(P, dtype=np.float32).astype(_BF16)

    in_maps = []
    for core in range(N_CORES):
        b, hg = core // 2, core % 2
        hs = slice(hg * NH, (hg + 1) * NH)
        xt = np.ascontiguousarray(x[b].T).astype(_BF16)           # [E, S]
        wq = np.ascontiguousarray(w[:, hs, 0, :].reshape(E, 512)).astype(_BF16)
        wk = np.ascontiguousarray(w[:, hs, 1, :].reshape(E, 512)).astype(_BF16)
        wv = np.ascontiguousarray(w[:, hs, 2, :].reshape(E, 512)).astype(_BF16)
        wo = np.ascontiguousarray(W_out[hg * 512 : (hg + 1) * 512, :]).astype(_BF16)
        bq = np.ascontiguousarray(b3[hs, 0, :].reshape(DT, P).T).astype(np.float32)
        bk = np.ascontiguousarray(b3[hs, 1, :].reshape(DT, P).T).astype(np.float32)
        bv = np.ascontiguousarray(b3[hs, 2, :].reshape(1, 512)).astype(_BF16)
        bo = (np.ascontiguousarray(b_out.reshape(ET, P).T) * (1.0 if hg == 0 else 0.0)).astype(np.float32)
        in_maps.append(
            {
                "xt": xt,
                "wq": wq,
                "wk": wk,
                "wv": wv,
                "wo": wo,
                "bq": bq,
                "bk": bk,
                "bv": bv,
                "bo": bo,
                "iden": iden,
            }
        )
    return in_maps


def run_raw(x, W_qkv, b_qkv, W_out, b_out, trace=False, **kw):
    """Run on hardware; returns (full_output [B,S,E] f32, BassKernelResults)."""
    global _cached
    from concourse.bass_utils import run_bass_kernel_spmd

    if _cached is None:
        _cached = _build()
    nc = _cached
    in_maps = _prep_inputs(
        np.asarray(x), np.asarray(W_qkv), np.asarray(b_qkv),
        np.asarray(W_out), np.asarray(b_out),
    )
    res = run_bass_kernel_spmd(
        nc, in_maps, core_ids=list(range(N_CORES)), trace=trace, **kw
    )
    out = np.empty((B, S, E), dtype=np.float32)
    for b in range(B):
        acc = np.asarray(res.results[2 * b]["out"]) + np.asarray(
            res.results[2 * b + 1]["out"]
        )
        out[b] = acc.T
    return out, res


def kernel(x, W_qkv, b_qkv, W_out, b_out):
    out, _ = run_raw(x, W_qkv, b_qkv, W_out, b_out, trace=False)
    return out


# revision 33
# speedup vs baseline: 1.4153x; 1.0212x over previous
"""Multi-head attention (B=4, S=2048, E=1024, H=16) on 8 TRN2 NeuronCores.

Sharding: batch x head-group tensor parallel -- core c = 2*b + hg handles
batch b and heads hg*8 .. hg*8+7 for ALL 2048 queries.  Q/K/V projections
are column-split by head (each core projects only its 8 heads); the output
projection is row-split (each core contracts its 512 E-rows of W_out) and
produces a partial [E, S] output that the HOST sums across the core pair
while unsharding (the "all-reduce" of the sharding hint, done on host).

Per-core kernel:
  - Q^T/K^T projections (bf16 matmul, fp32 PSUM) evacuated with fused
    bias-add + fp8e4 quantization (DVE tensor_scalar_add, fp8 out).
  - scores via fp8 DoubleRow matmuls: contraction d=64 fed as
    [64 part, 2(dup, stride 0), N]; the duplicated group doubles the
    result and the exp activation scale absorbs the factor 2.
    Cost: 0.5 cycles/row (vs 1.0 bf16).
  - exp on ScalarE (the bottleneck engine, ~266us busy): one [128, 1024]
    activation per key-tile j, reading two adjacent 512-wide slots of a
    manual 4-slot PSUM ring (slots 2j%4, 2j%4+1 -> flat AP; the two slot
    pairs double-buffer).  Scores for j+1 are emitted BEFORE the PV
    matmuls of j so the exp stream never waits on PE's in-order queue.
  - PV in the FLIPPED orientation: out[q=128, 65] = P_tile.T @ [V | ones]
    (all 128 output partitions vs 65 the naive way); the ones column is
    the softmax denominator per query row.  The 8 per-qt accumulators
    live in one [128, 8, 128] tile (qt stride 512B -> no bank crossing);
    PSUM start=True zero-fills a whole 2KB region, so only the first
    matmul touching each bank uses start=True and the rest rely on the
    pending-zero overwrite semantics (no memset needed).
  - normalization: per-partition reciprocal of the denominator column +
    tensor_scalar multiply -> O in [q, d]; PE-transpose (identity
    matmul) back to O^T for the out projection.

Schedule: 2 query-half phase groups x 8 heads x 16 key tiles (j).
Per j-step the PE also runs one or two small interleaved chunks: V
projection for the NEXT head (just-in-time, ~240ns each), K/Q
projection half-chunks (~850ns), O^T transposes, and the first half's
output projection (during the second half).  Only the second half's
output projection is a serial tail.
"""

import sys

if "/opt/trn_rl_repo" not in sys.path:
    sys.path.insert(0, "/opt/trn_rl_repo")

import numpy as np
import ml_dtypes

B, S, E, H = 4, 2048, 1024, 16
P = 128
HD = 64           # head dim
NH = 8            # heads per core
DT = 4            # d-tiles (head pairs) per core
ET = E // P       # 8 e-tiles (contraction for projections)
ST = S // P       # 16 key tiles
N_CORES = 8
QH = S // 2       # query half width (1024)
SCALE = 1.0 / float(np.sqrt(HD))

_BF16 = ml_dtypes.bfloat16

_cached = None


def _build():
    import concourse.bass as bass
    import concourse.tile as tile
    import concourse.mybir as mybir
    from concourse import bacc

    dt = mybir.dt
    nc = bacc.Bacc("TRN2", target_bir_lowering=False, debug=False)

    xt_d = nc.dram_tensor("xt", [E, S], dt.bfloat16, kind="ExternalInput").ap()
    wq_d = nc.dram_tensor("wq", [E, 512], dt.bfloat16, kind="ExternalInput").ap()
    wk_d = nc.dram_tensor("wk", [E, 512], dt.bfloat16, kind="ExternalInput").ap()
    wv_d = nc.dram_tensor("wv", [E, 512], dt.bfloat16, kind="ExternalInput").ap()
    wo_d = nc.dram_tensor("wo", [512, E], dt.bfloat16, kind="ExternalInput").ap()
    bq_d = nc.dram_tensor("bq", [P, DT], dt.float32, kind="ExternalInput").ap()
    bk_d = nc.dram_tensor("bk", [P, DT], dt.float32, kind="ExternalInput").ap()
    bv_d = nc.dram_tensor("bv", [1, 512], dt.bfloat16, kind="ExternalInput").ap()
    bo_d = nc.dram_tensor("bo", [P, ET], dt.float32, kind="ExternalInput").ap()
    iden_d = nc.dram_tensor("iden", [P, P], dt.bfloat16, kind="ExternalInput").ap()
    out_d = nc.dram_tensor("out", [E, S], dt.float32, kind="ExternalOutput").ap()

    DR = mybir.MatmulPerfMode.DoubleRow

    with tile.TileContext(nc) as tc:
        with (
            tc.tile_pool(name="const", bufs=1) as cpool,
            tc.tile_pool(name="acts", bufs=1) as apool,
            tc.tile_pool(name="pp", bufs=3) as ppool,        # P (exp out)
            tc.tile_pool(name="oqp", bufs=2) as oqpool,      # O [q, dd] staging
            tc.tile_pool(name="recp", bufs=2) as recpool,    # reciprocals
            tc.tile_pool(name="outs", bufs=8) as outpool,    # out staging
            tc.tile_pool(name="pssc", bufs=2, space="PSUM") as scpool,   # 4 banks
            tc.tile_pool(name="pspv", bufs=1, space="PSUM") as pvpool,   # 2 banks
            tc.tile_pool(name="pspj", bufs=1, space="PSUM") as pjpool,   # 1 bank
            tc.tile_pool(name="pstp", bufs=1, space="PSUM") as tppool,   # 1 bank
        ):
            # ---------------- constants / inputs -----------------------
            xt = cpool.tile([P, ET, S], dt.bfloat16)
            wq = cpool.tile([P, ET, 512], dt.bfloat16)
            wk = cpool.tile([P, ET, 512], dt.bfloat16)
            wv = cpool.tile([P, ET, 512], dt.bfloat16)
            wo = cpool.tile([P, DT, E], dt.bfloat16)
            bq = cpool.tile([P, DT], dt.float32)
            bk = cpool.tile([P, DT], dt.float32)
            bv = cpool.tile([1, 512], dt.bfloat16)
            bo = cpool.tile([P, ET], dt.float32)
            iden = cpool.tile([P, P], dt.bfloat16)
            ones1 = cpool.tile([1, P], dt.bfloat16)

            # activations
            qt8 = apool.tile([P, DT, S], dt.float8e4)   # Q^T (bias+fp8)
            kt8 = apool.tile([P, DT, S], dt.float8e4)   # K^T (bias+fp8)
            va = apool.tile([P, ST, NH, HD + 1], dt.bfloat16)  # V | ones
            scb = apool.tile([P, DT, S], dt.bfloat16)   # O^T (normalized)

            # Long-lived PSUM tiles.  Dependency tracking is TILE-granular,
            # so the scores ring is TWO alternating tiles: exp(step) then
            # only depends on its own tile's scores, and scores(step+2)
            # (same tile) WAR-waits exp(step) -- the other tile streams
            # freely underneath.
            rings = [
                scpool.tile([P, 2, 512], dt.float32, tag="sc", name="ringA"),
                scpool.tile([P, 2, 512], dt.float32, tag="sc", name="ringB"),
            ]
            pv = pvpool.tile([P, 8, P], dt.float32, tag="pv", name="pv")

            # DMA transfers serialize on the global DMA device, so order by
            # criticality: d-tile-0 K/Q slices and head-0 V cols first, the
            # full x^T (bandwidth floor ~12.6us), then everything else.
            wkr = wk_d.rearrange("(eo p) c -> p eo c", p=P)
            wqr = wq_d.rearrange("(eo p) c -> p eo c", p=P)
            wvr = wv_d.rearrange("(eo p) c -> p eo c", p=P)
            nc.sync.dma_start(wk[:, :, 0:P], wkr[:, :, 0:P])
            nc.sync.dma_start(bk[:], bk_d)
            for e in range(ET):
                nc.sync.dma_start(xt[:, e, :], xt_d[e * P : (e + 1) * P, :])
            nc.sync.dma_start(wq[:, :, 0:P], wqr[:, :, 0:P])
            nc.sync.dma_start(bq[:], bq_d)
            nc.sync.dma_start(wv[:, :, 0:HD], wvr[:, :, 0:HD])
            nc.sync.dma_start(bv[:], bv_d)
            nc.sync.dma_start(wk[:, :, P:512], wkr[:, :, P:512])
            nc.sync.dma_start(wq[:, :, P:512], wqr[:, :, P:512])
            nc.sync.dma_start(wv[:, :, HD:512], wvr[:, :, HD:512])
            nc.sync.dma_start(iden[:], iden_d)
            nc.sync.dma_start(wo[:], wo_d.rearrange("(eo p) c -> p eo c", p=P))
            nc.sync.dma_start(bo[:], bo_d)
            nc.gpsimd.memset(ones1[:], 1.0)
            nc.gpsimd.memset(va[:, :, :, HD : HD + 1], 1.0)

            # ---------------- small-chunk emitters ----------------------
            # kq/outproj chunks come as (partA, partB) sharing one psum tile;
            # with a single-buffer proj pool they are also adjacent-safe.

            def kq_halves(which, t, c):
                w_, b_, dst = (wk, bk, kt8) if which == "k" else (wq, bq, qt8)
                holder = {}

                def part0():
                    ps = pjpool.tile(
                        [P, 512], dt.float32, tag="pj", name=f"{which}{t}{c}"
                    )
                    holder[0] = ps
                    for e in range(4):
                        nc.tensor.matmul(
                            ps[:],
                            w_[:, e, t * P : (t + 1) * P],
                            xt[:, e, c * 512 : (c + 1) * 512],
                            start=(e == 0),
                            stop=False,
                        )

                def part1():
                    ps = holder[0]
                    for e in range(4, ET):
                        nc.tensor.matmul(
                            ps[:],
                            w_[:, e, t * P : (t + 1) * P],
                            xt[:, e, c * 512 : (c + 1) * 512],
                            start=False,
                            stop=(e == ET - 1),
                        )
                    nc.vector.tensor_scalar_add(
                        dst[:, t, c * 512 : (c + 1) * 512], ps[:], b_[:, t : t + 1]
                    )

                return [part0, part1]

            def v_chunk(h, st):
                """V rows for (head h, key tile st): [128 keys, 64] + bias.
                Uses the transpose-psum bank (tiny tiles) to stay off the
                kq/outproj pipeline."""
                def go():
                    ps = tppool.tile([P, P], dt.float32, tag="vps", name=f"v{h}{st}")
                    for e in range(ET):
                        nc.tensor.matmul(
                            ps[:, 0:HD],
                            xt[:, e, st * P : (st + 1) * P],
                            wv[:, e, h * HD : (h + 1) * HD],
                            start=(e == 0),
                            stop=False,
                        )
                    nc.tensor.matmul(
                        ps[:, 0:HD],
                        ones1[0:1, :],
                        bv[0:1, h * HD : (h + 1) * HD],
                        start=False,
                        stop=True,
                    )
                    nc.vector.tensor_copy(va[:, st, h, 0:HD], ps[:, 0:HD])
                return go

            def outproj_halves(eo, qq):
                """Partial out^T tile [128 Eo, 512 q] for query quarter qq."""
                holder = {}
                q0 = qq * 512

                def part0():
                    ps = pjpool.tile([P, 512], dt.float32, tag="pj", name=f"o{eo}{qq}")
                    holder[0] = ps
                    for t in (0, 1):
                        nc.tensor.matmul(
                            ps[:],
                            wo[:, t, eo * P : (eo + 1) * P],
                            scb[:, t, q0 : q0 + 512],
                            start=(t == 0),
                            stop=False,
                        )

                def part1():
                    ps = holder[0]
                    for t in (2, 3):
                        nc.tensor.matmul(
                            ps[:],
                            wo[:, t, eo * P : (eo + 1) * P],
                            scb[:, t, q0 : q0 + 512],
                            start=False,
                            stop=(t == DT - 1),
                        )
                    ot = outpool.tile(
                        [P, 512], dt.float32, tag="ot", name=f"oe{eo}{qq}"
                    )
                    nc.vector.tensor_scalar_add(ot[:], ps[:], bo[:, eo : eo + 1])
                    nc.sync.dma_start(
                        out_d[eo * P : (eo + 1) * P, q0 : q0 + 512], ot[:]
                    )

                return [part0, part1]

            oq_tiles = {}

            def transpose_item(t, qh, qt):
                """oq [q, dd of pair t] -> scb[:, t, ...] via PE transpose.
                Uses the same psum bank as v_chunk (different tag would
                double-book the bank, so share tag/shape via bitcast)."""
                def go():
                    tp = tppool.tile([P, P], dt.float32, tag="vps", name=f"tp{t}{qh}{qt}")
                    tpb = tp[:, 0:HD].bitcast(dt.bfloat16)
                    nc.tensor.transpose(tpb, oq_tiles[(t, qh)][:, qt, :], iden[:])
                    q0 = qh * QH + qt * P
                    nc.vector.tensor_copy(scb[:, t, q0 : q0 + P], tpb)
                return go

            # ---------------- attention stream ---------------------------
            # One global stream of 256 j-steps (16 phases x 16 key tiles).
            # Per step: exp(step) on ACT, then on PE scores(step+2) (its ring
            # slots were just freed by exp(step)), then the PV wave of step,
            # then interleaved thunks.  This keeps the serial chain between
            # consecutive exps down to one sem hop, across phase boundaries
            # included.
            def scores(step):
                h, qh, j = phase_of(step)
                t, hp = h // 2, (h % 2) * HD
                q0 = qh * QH
                rg = rings[step % 2]
                for qc in range(2):
                    nc.tensor.matmul(
                        rg[:, qc, :],
                        kt8[hp : hp + HD, t, j * P : (j + 1) * P]
                        .unsqueeze(1)
                        .broadcast_to((HD, 2, P)),
                        qt8[hp : hp + HD, t, q0 + qc * 512 : q0 + (qc + 1) * 512]
                        .unsqueeze(1)
                        .broadcast_to((HD, 2, 512)),
                        start=True,
                        stop=True,
                        perf_mode=DR,
                    )

            def phase_of(step):
                phase, j = divmod(step, ST)
                qh, h = divmod(phase, NH)
                return h, qh, j

            def emit_evac(h, qh):
                t, half = h // 2, h % 2
                if half == 0:
                    oq_tiles[(t, qh)] = oqpool.tile(
                        [P, 8, P], dt.bfloat16, tag="oq", name=f"oq{t}{qh}"
                    )
                oq = oq_tiles[(t, qh)]
                rec = recpool.tile([P, 8], dt.float32, tag="rec", name=f"rc{h}{qh}")
                scr = recpool.tile([P, 8], dt.float32, tag="scr", name=f"sr{h}{qh}")
                nc.vector.reciprocal_approx_accurate(
                    rec[:], pv[:, :, HD : HD + 1].rearrange("p a b -> p (a b)"), scr[:]
                )
                for qt in range(8):
                    nc.vector.tensor_scalar(
                        oq[:, qt, half * HD : (half + 1) * HD],
                        pv[:, qt, 0:HD],
                        rec[:, qt : qt + 1],
                        None,
                        op0=mybir.AluOpType.mult,
                    )

            def run_stream(work):
                """work: list of dicts {release, due, fns: [(fn, cost), ...]}.
                Per step, spend ~STEP_BUDGET ns of PE time on the earliest-due
                released items; a partially-emitted item always continues
                first (its parts share one psum tile)."""
                STEP_BUDGET = 620.0
                n_steps = 16 * NH * 2
                scores(0)
                scores(1)
                pending = sorted(work, key=lambda w: (w["due"], w["release"]))
                current = None
                for step in range(n_steps):
                    h, qh, j = phase_of(step)
                    ptile = ppool.tile(
                        [P, 2, 512], dt.bfloat16, tag="p", name=f"p{step}"
                    )
                    nc.scalar.activation(
                        ptile.rearrange("p a b -> p (a b)"),
                        rings[step % 2].rearrange("p a b -> p (a b)"),
                        mybir.ActivationFunctionType.Exp,
                        scale=SCALE / 2.0,
                    )
                    if step + 2 < n_steps:
                        scores(step + 2)
                    for qt in range(8):
                        nc.tensor.matmul(
                            pv[:, qt, 0 : HD + 1],
                            ptile[:, qt // 4, (qt % 4) * P : (qt % 4 + 1) * P],
                            va[:, j, h, :],
                            start=(j == 0 and qt % 4 == 0),
                            stop=(j == ST - 1),
                            skip_group_check=True,
                        )
                    budget = STEP_BUDGET
                    while budget > 0:
                        if current is None:
                            cand = [w for w in pending if w["release"] <= step]
                            if not cand:
                                break
                            current = cand[0]
                            pending.remove(current)
                            assert current["due"] >= step, (
                                f"work item overdue: emitted step {step}, "
                                f"due {current['due']}"
                            )
                        fn, cost = current["fns"].pop(0)
                        fn()
                        budget -= cost
                        if not current["fns"]:
                            current = None
                    if j == ST - 1:
                        emit_evac(h, qh)
                # anything left (tail work: release >= n_steps)
                leftovers = ([current] if current else []) + pending
                leftovers.sort(key=lambda w: (w["release"], w["due"]))
                for w in leftovers:
                    for fn, _ in w["fns"]:
                        fn()

            # ---------------- emission schedule -------------------------
            # Pre-attention prefix: K/Q d-tile 0 for the first half, with
            # V chunks (separate psum bank) filling the proj-evac gaps.
            prefix_kq = (
                kq_halves("k", 0, 0) + kq_halves("q", 0, 0)
                + kq_halves("k", 0, 1) + kq_halves("q", 0, 1)
            )
            prefix_v = [v_chunk(0, st) for st in range(6)]
            for i, fn in enumerate(prefix_kq):
                fn()
                if i % 2 == 1 and prefix_v:
                    prefix_v.pop(0)()
            for fn in prefix_v:
                fn()

            # Work items with release/due steps.  Due dates: a K/Q chunk
            # feeding scores(x) must finish by step x-3 (scores run two
            # steps ahead and precede thunks within a step); a V chunk
            # feeding PV(x) by step x-1.
            KQC, VC, TRC, OPC = 950.0, 340.0, 250.0, 650.0
            work = []

            def add(release, due, fns, cost):
                work.append(
                    {"release": release, "due": due,
                     "fns": [(f, cost) for f in fns]}
                )

            # V: head 0 tiles 6..15 (0..5 in the prefix), then all other heads
            for st in range(6, ST):
                add(0, st - 1, [v_chunk(0, st)], VC)
            for h in range(1, NH):
                for st in range(ST):
                    add(0, 16 * h + st - 1, [v_chunk(h, st)], VC)
            # K: d-tile 0 chunks 2/3 (0/1 in the prefix), d-tiles 1..3 all
            for c in (2, 3):
                add(0, 4 * c - 3, kq_halves("k", 0, c), KQC / 2)
            for t2 in range(1, DT):
                for c in range(4):
                    add(0, 32 * t2 + 4 * c - 3, kq_halves("k", t2, c), KQC / 2)
            # Q: low half (chunks 0/1) due at (2t, qh0); high half at qh1
            for t2 in range(1, DT):
                for c in (0, 1):
                    add(0, 32 * t2 - 3, kq_halves("q", t2, c), KQC / 2)
            for t2 in range(DT):
                for c in (2, 3):
                    add(0, 128 + 32 * t2 - 3, kq_halves("q", t2, c), KQC / 2)
            # transposes: tight due dates so they spread right after their
            # pair completes (the single transpose buffer serializes
            # clumped transposes at ~450ns each)
            for t2 in range(DT):
                rel = 32 * t2 + 32
                add(rel, rel + 28 if t2 < DT - 1 else 145,
                    [transpose_item(t2, 0, qt) for qt in range(8)], TRC)
            for t2 in range(DT):
                rel = 176 + 32 * t2
                add(rel, rel + 28 if t2 < DT - 1 else 10**6,
                    [transpose_item(t2, 1, qt) for qt in range(8)], TRC)
            # outproj quarters 0/1 during half 1; quarters 2/3 in the tail
            for eo in range(ET):
                for qq in (0, 1):
                    add(146, 254, outproj_halves(eo, qq), OPC)
            # tail outproj uses the (then idle) scores ring as rotating psum
            ci = [0]

            def tail_outproj(eo, qq):
                def go():
                    ps = rings[ci[0] % 2][:, (ci[0] // 2) % 2, :]
                    ci[0] += 1
                    q0 = qq * 512
                    for t in range(DT):
                        nc.tensor.matmul(
                            ps,
                            wo[:, t, eo * P : (eo + 1) * P],
                            scb[:, t, q0 : q0 + 512],
                            start=(t == 0),
                            stop=(t == DT - 1),
                        )
                    ot = outpool.tile(
                        [P, 512], dt.float32, tag="ot", name=f"ot{eo}{qq}"
                    )
                    nc.vector.tensor_scalar_add(ot[:], ps, bo[:, eo : eo + 1])
                    nc.sync.dma_start(
                        out_d[eo * P : (eo + 1) * P, q0 : q0 + 512], ot[:]
                    )
                return go

            for eo in range(ET):
                for qq in (2, 3):
                    add(10**6, 10**6, [tail_outproj(eo, qq)], OPC)

            # Run the whole attention stream (tail work included).
            run_stream(work)

    nc.compile()
    return nc


def _prep_inputs(x, W_qkv, b_qkv, W_out, b_out):
    """Host-side sharding + layout prep. Returns per-core input maps."""
    w = W_qkv.reshape(E, H, 3, HD)
    b3 = b_qkv.reshape(H, 3, HD)
    iden = np.eye(P, dtype=np.float32).astype(_BF16)

    in_maps = []
    for core in range(N_CORES):
        b, hg = core // 2, core % 2
        hs = slice(hg * NH, (hg + 1) * NH)
        xt = np.ascontiguousarray(x[b].T).astype(_BF16)           # [E, S]
        wq = np.ascontiguousarray(w[:, hs, 0, :].reshape(E, 512)).astype(_BF16)
        wk = np.ascontiguousarray(w[:, hs, 1, :].reshape(E, 512)).astype(_BF16)
        wv = np.ascontiguousarray(w[:, hs, 2, :].reshape(E, 512)).astype(_BF16)
        wo = np.ascontiguousarray(W_out[hg * 512 : (hg + 1) * 512, :]).astype(_BF16)
        bq = np.ascontiguousarray(b3[hs, 0, :].reshape(DT, P).T).astype(np.float32)
        bk = np.ascontiguousarray(b3[hs, 1, :].reshape(DT, P).T).astype(np.float32)
        bv = np.ascontiguousarray(b3[hs, 2, :].reshape(1, 512)).astype(_BF16)
        bo = (np.ascontiguousarray(b_out.reshape(ET, P).T) * (1.0 if hg == 0 else 0.0)).astype(np.float32)
        in_maps.append(
            {
                "xt": xt,
                "wq": wq,
                "wk": wk,
                "wv": wv,
                "wo": wo,
                "bq": bq,
                "bk": bk,
                "bv": bv,
                "bo": bo,
                "iden": iden,
            }
        )
    return in_maps


def run_raw(x, W_qkv, b_qkv, W_out, b_out, trace=False, **kw):
    """Run on hardware; returns (full_output [B,S,E] f32, BassKernelResults)."""
    global _cached
    from concourse.bass_utils import run_bass_kernel_spmd

    if _cached is None:
        _cached = _build()
    nc = _cached
    in_maps = _prep_inputs(
        np.asarray(x), np.asarray(W_qkv), np.asarray(b_qkv),
        np.asarray(W_out), np.asarray(b_out),
    )
    res = run_bass_kernel_spmd(
        nc, in_maps, core_ids=list(range(N_CORES)), trace=trace, **kw
    )
    out = np.empty((B, S, E), dtype=np.float32)
    for b in range(B):
        acc = np.asarray(res.results[2 * b]["out"]) + np.asarray(
            res.results[2 * b + 1]["out"]
        )
        out[b] = acc.T
    return out, res


def kernel(x, W_qkv, b_qkv, W_out, b_out):
    out, _ = run_raw(x, W_qkv, b_qkv, W_out, b_out, trace=False)
    return out


# revision 35
# speedup vs baseline: 1.4361x; 1.0147x over previous
"""Multi-head attention (B=4, S=2048, E=1024, H=16) on 8 TRN2 NeuronCores.

Sharding: batch x head-group tensor parallel -- core c = 2*b + hg handles
batch b and heads hg*8 .. hg*8+7 for ALL 2048 queries.  Q/K/V projections
are column-split by head (each core projects only its 8 heads); the output
projection is row-split (each core contracts its 512 E-rows of W_out) and
produces a partial [E, S] output that the HOST sums across the core pair
while unsharding (the "all-reduce" of the sharding hint, done on host).

Per-core kernel:
  - Q^T/K^T projections (bf16 matmul, fp32 PSUM) evacuated with fused
    bias-add + fp8e4 quantization (DVE tensor_scalar_add, fp8 out).
  - scores via fp8 DoubleRow matmuls: contraction d=64 fed as
    [64 part, 2(dup, stride 0), N]; the duplicated group doubles the
    result and the exp activation scale absorbs the factor 2.
    Cost: 0.5 cycles/row (vs 1.0 bf16).
  - exp on ScalarE (the bottleneck engine, ~266us busy): one [128, 1024]
    activation per key-tile j, reading two adjacent 512-wide slots of a
    manual 4-slot PSUM ring (slots 2j%4, 2j%4+1 -> flat AP; the two slot
    pairs double-buffer).  Scores for j+1 are emitted BEFORE the PV
    matmuls of j so the exp stream never waits on PE's in-order queue.
  - PV in the FLIPPED orientation: out[q=128, 65] = P_tile.T @ [V | ones]
    (all 128 output partitions vs 65 the naive way); the ones column is
    the softmax denominator per query row.  The 8 per-qt accumulators
    live in one [128, 8, 128] tile (qt stride 512B -> no bank crossing);
    PSUM start=True zero-fills a whole 2KB region, so only the first
    matmul touching each bank uses start=True and the rest rely on the
    pending-zero overwrite semantics (no memset needed).
  - normalization: per-partition reciprocal of the denominator column +
    tensor_scalar multiply -> O in [q, d]; PE-transpose (identity
    matmul) back to O^T for the out projection.

Schedule: 2 query-half phase groups x 8 heads x 16 key tiles (j).
Per j-step the PE also runs one or two small interleaved chunks: V
projection for the NEXT head (just-in-time, ~240ns each), K/Q
projection half-chunks (~850ns), O^T transposes, and the first half's
output projection (during the second half).  Only the second half's
output projection is a serial tail.
"""

import sys

if "/opt/trn_rl_repo" not in sys.path:
    sys.path.insert(0, "/opt/trn_rl_repo")

import numpy as np
import ml_dtypes

B, S, E, H = 4, 2048, 1024, 16
P = 128
HD = 64           # head dim
NH = 8            # heads per core
DT = 4            # d-tiles (head pairs) per core
ET = E // P       # 8 e-tiles (contraction for projections)
ST = S // P       # 16 key tiles
N_CORES = 8
QH = S // 2       # query half width (1024)
SCALE = 1.0 / float(np.sqrt(HD))

_BF16 = ml_dtypes.bfloat16

_cached = None


def _build():
    import concourse.bass as bass
    import concourse.tile as tile
    import concourse.mybir as mybir
    from concourse import bacc

    dt = mybir.dt
    nc = bacc.Bacc("TRN2", target_bir_lowering=False, debug=False)

    xt_d = nc.dram_tensor("xt", [E, S], dt.bfloat16, kind="ExternalInput").ap()
    wq_d = nc.dram_tensor("wq", [E, 512], dt.bfloat16, kind="ExternalInput").ap()
    wk_d = nc.dram_tensor("wk", [E, 512], dt.bfloat16, kind="ExternalInput").ap()
    wv_d = nc.dram_tensor("wv", [E, 512], dt.bfloat16, kind="ExternalInput").ap()
    wo_d = nc.dram_tensor("wo", [512, E], dt.bfloat16, kind="ExternalInput").ap()
    bq_d = nc.dram_tensor("bq", [P, DT], dt.float32, kind="ExternalInput").ap()
    bk_d = nc.dram_tensor("bk", [P, DT], dt.float32, kind="ExternalInput").ap()
    bv_d = nc.dram_tensor("bv", [1, 512], dt.bfloat16, kind="ExternalInput").ap()
    bo_d = nc.dram_tensor("bo", [P, ET], dt.float32, kind="ExternalInput").ap()
    iden_d = nc.dram_tensor("iden", [P, P], dt.bfloat16, kind="ExternalInput").ap()
    out_d = nc.dram_tensor("out", [E, S], dt.float32, kind="ExternalOutput").ap()

    DR = mybir.MatmulPerfMode.DoubleRow

    with tile.TileContext(nc) as tc:
        with (
            tc.tile_pool(name="const", bufs=1) as cpool,
            tc.tile_pool(name="acts", bufs=1) as apool,
            tc.tile_pool(name="pp", bufs=3) as ppool,        # P (exp out)
            tc.tile_pool(name="oqp", bufs=2) as oqpool,      # O [q, dd] staging
            tc.tile_pool(name="recp", bufs=2) as recpool,    # reciprocals
            tc.tile_pool(name="outs", bufs=8) as outpool,    # out staging
            tc.tile_pool(name="pssc", bufs=2, space="PSUM") as scpool,   # 4 banks
            tc.tile_pool(name="pspv", bufs=1, space="PSUM") as pvpool,   # 2 banks
            tc.tile_pool(name="pspj", bufs=1, space="PSUM") as pjpool,   # 1 bank
            tc.tile_pool(name="pstp", bufs=1, space="PSUM") as tppool,   # 1 bank
        ):
            # ---------------- constants / inputs -----------------------
            xt = cpool.tile([P, ET, S], dt.bfloat16)
            wq = cpool.tile([P, ET, 512], dt.bfloat16)
            wk = cpool.tile([P, ET, 512], dt.bfloat16)
            wv = cpool.tile([P, ET, 512], dt.bfloat16)
            wo = cpool.tile([P, DT, E], dt.bfloat16)
            bq = cpool.tile([P, DT], dt.float32)
            bk = cpool.tile([P, DT], dt.float32)
            bv = cpool.tile([1, 512], dt.bfloat16)
            bo = cpool.tile([P, ET], dt.float32)
            iden = cpool.tile([P, P], dt.bfloat16)
            ones1 = cpool.tile([1, P], dt.bfloat16)

            # activations
            qt8 = apool.tile([P, DT, S], dt.float8e4)   # Q^T (bias+fp8)
            kt8 = apool.tile([P, DT, S], dt.float8e4)   # K^T (bias+fp8)
            va = apool.tile([P, ST, NH, HD + 1], dt.bfloat16)  # V | ones
            scb = apool.tile([P, DT, S], dt.bfloat16)   # O^T (normalized)

            # Long-lived PSUM tiles.  Dependency tracking is TILE-granular,
            # so the scores ring is TWO alternating tiles: exp(step) then
            # only depends on its own tile's scores, and scores(step+2)
            # (same tile) WAR-waits exp(step) -- the other tile streams
            # freely underneath.
            rings = [
                scpool.tile([P, 2, 512], dt.float32, tag="sc", name="ringA"),
                scpool.tile([P, 2, 512], dt.float32, tag="sc", name="ringB"),
            ]
            pv = pvpool.tile([P, 8, P], dt.float32, tag="pv", name="pv")

            # DMA transfers serialize on the global DMA device, so order by
            # criticality: d-tile-0 K/Q slices and head-0 V cols first, the
            # full x^T (bandwidth floor ~12.6us), then everything else.
            wkr = wk_d.rearrange("(eo p) c -> p eo c", p=P)
            wqr = wq_d.rearrange("(eo p) c -> p eo c", p=P)
            wvr = wv_d.rearrange("(eo p) c -> p eo c", p=P)
            nc.sync.dma_start(wk[:, :, 0:P], wkr[:, :, 0:P])
            nc.sync.dma_start(wq[:, :, 0:P], wqr[:, :, 0:P])
            nc.sync.dma_start(wv[:, :, 0:HD], wvr[:, :, 0:HD])
            nc.sync.dma_start(bk[:], bk_d)
            nc.sync.dma_start(bq[:], bq_d)
            nc.sync.dma_start(bv[:], bv_d)
            for e in range(ET):
                nc.sync.dma_start(xt[:, e, :], xt_d[e * P : (e + 1) * P, :])
            nc.sync.dma_start(wk[:, :, P:512], wkr[:, :, P:512])
            nc.sync.dma_start(wq[:, :, P:512], wqr[:, :, P:512])
            nc.sync.dma_start(wv[:, :, HD:512], wvr[:, :, HD:512])
            nc.sync.dma_start(iden[:], iden_d)
            nc.sync.dma_start(wo[:], wo_d.rearrange("(eo p) c -> p eo c", p=P))
            nc.sync.dma_start(bo[:], bo_d)
            nc.gpsimd.memset(ones1[:], 1.0)
            nc.gpsimd.memset(va[:, :, :, HD : HD + 1], 1.0)

            # ---------------- small-chunk emitters ----------------------
            # kq/outproj chunks come as (partA, partB) sharing one psum tile;
            # with a single-buffer proj pool they are also adjacent-safe.

            def kq_halves(which, t, c):
                w_, b_, dst = (wk, bk, kt8) if which == "k" else (wq, bq, qt8)
                holder = {}

                def part0():
                    ps = pjpool.tile(
                        [P, 512], dt.float32, tag="pj", name=f"{which}{t}{c}"
                    )
                    holder[0] = ps
                    for e in range(4):
                        nc.tensor.matmul(
                            ps[:],
                            w_[:, e, t * P : (t + 1) * P],
                            xt[:, e, c * 512 : (c + 1) * 512],
                            start=(e == 0),
                            stop=False,
                        )

                def part1():
                    ps = holder[0]
                    for e in range(4, ET):
                        nc.tensor.matmul(
                            ps[:],
                            w_[:, e, t * P : (t + 1) * P],
                            xt[:, e, c * 512 : (c + 1) * 512],
                            start=False,
                            stop=(e == ET - 1),
                        )
                    nc.vector.tensor_scalar_add(
                        dst[:, t, c * 512 : (c + 1) * 512], ps[:], b_[:, t : t + 1]
                    )

                return [part0, part1]

            def v_chunk(h, st):
                """V rows for (head h, key tile st): [128 keys, 64] + bias.
                Uses the transpose-psum bank (tiny tiles) to stay off the
                kq/outproj pipeline."""
                def go():
                    ps = tppool.tile([P, P], dt.float32, tag="vps", name=f"v{h}{st}")
                    for e in range(ET):
                        nc.tensor.matmul(
                            ps[:, 0:HD],
                            xt[:, e, st * P : (st + 1) * P],
                            wv[:, e, h * HD : (h + 1) * HD],
                            start=(e == 0),
                            stop=False,
                        )
                    nc.tensor.matmul(
                        ps[:, 0:HD],
                        ones1[0:1, :],
                        bv[0:1, h * HD : (h + 1) * HD],
                        start=False,
                        stop=True,
                    )
                    nc.vector.tensor_copy(va[:, st, h, 0:HD], ps[:, 0:HD])
                return go

            def outproj_halves(eo, qq):
                """Partial out^T tile [128 Eo, 512 q] for query quarter qq."""
                holder = {}
                q0 = qq * 512

                def part0():
                    ps = pjpool.tile([P, 512], dt.float32, tag="pj", name=f"o{eo}{qq}")
                    holder[0] = ps
                    for t in (0, 1):
                        nc.tensor.matmul(
                            ps[:],
                            wo[:, t, eo * P : (eo + 1) * P],
                            scb[:, t, q0 : q0 + 512],
                            start=(t == 0),
                            stop=False,
                        )

                def part1():
                    ps = holder[0]
                    for t in (2, 3):
                        nc.tensor.matmul(
                            ps[:],
                            wo[:, t, eo * P : (eo + 1) * P],
                            scb[:, t, q0 : q0 + 512],
                            start=False,
                            stop=(t == DT - 1),
                        )
                    ot = outpool.tile(
                        [P, 512], dt.float32, tag="ot", name=f"oe{eo}{qq}"
                    )
                    nc.vector.tensor_scalar_add(ot[:], ps[:], bo[:, eo : eo + 1])
                    nc.sync.dma_start(
                        out_d[eo * P : (eo + 1) * P, q0 : q0 + 512], ot[:]
                    )

                return [part0, part1]

            oq_tiles = {}

            def transpose_item(t, qh, qt):
                """oq [q, dd of pair t] -> scb[:, t, ...] via PE transpose.
                Uses the same psum bank as v_chunk (different tag would
                double-book the bank, so share tag/shape via bitcast)."""
                def go():
                    tp = tppool.tile([P, P], dt.float32, tag="vps", name=f"tp{t}{qh}{qt}")
                    tpb = tp[:, 0:HD].bitcast(dt.bfloat16)
                    nc.tensor.transpose(tpb, oq_tiles[(t, qh)][:, qt, :], iden[:])
                    q0 = qh * QH + qt * P
                    nc.vector.tensor_copy(scb[:, t, q0 : q0 + P], tpb)
                return go

            # ---------------- attention stream ---------------------------
            # One global stream of 256 j-steps (16 phases x 16 key tiles).
            # Per step: exp(step) on ACT, then on PE scores(step+2) (its ring
            # slots were just freed by exp(step)), then the PV wave of step,
            # then interleaved thunks.  This keeps the serial chain between
            # consecutive exps down to one sem hop, across phase boundaries
            # included.
            def scores(step):
                h, qh, j = phase_of(step)
                t, hp = h // 2, (h % 2) * HD
                q0 = qh * QH
                rg = rings[step % 2]
                for qc in range(2):
                    nc.tensor.matmul(
                        rg[:, qc, :],
                        kt8[hp : hp + HD, t, j * P : (j + 1) * P]
                        .unsqueeze(1)
                        .broadcast_to((HD, 2, P)),
                        qt8[hp : hp + HD, t, q0 + qc * 512 : q0 + (qc + 1) * 512]
                        .unsqueeze(1)
                        .broadcast_to((HD, 2, 512)),
                        start=True,
                        stop=True,
                        perf_mode=DR,
                    )

            def phase_of(step):
                phase, j = divmod(step, ST)
                qh, h = divmod(phase, NH)
                return h, qh, j

            def emit_evac(h, qh):
                t, half = h // 2, h % 2
                if half == 0:
                    oq_tiles[(t, qh)] = oqpool.tile(
                        [P, 8, P], dt.bfloat16, tag="oq", name=f"oq{t}{qh}"
                    )
                oq = oq_tiles[(t, qh)]
                rec = recpool.tile([P, 8], dt.float32, tag="rec", name=f"rc{h}{qh}")
                scr = recpool.tile([P, 8], dt.float32, tag="scr", name=f"sr{h}{qh}")
                nc.vector.reciprocal_approx_accurate(
                    rec[:], pv[:, :, HD : HD + 1].rearrange("p a b -> p (a b)"), scr[:]
                )
                for qt in range(8):
                    nc.vector.tensor_scalar(
                        oq[:, qt, half * HD : (half + 1) * HD],
                        pv[:, qt, 0:HD],
                        rec[:, qt : qt + 1],
                        None,
                        op0=mybir.AluOpType.mult,
                    )

            def run_stream(work):
                """work: list of dicts {release, due, fns: [(fn, cost), ...]}.
                Per step, spend ~STEP_BUDGET ns of PE time on the earliest-due
                released items; a partially-emitted item always continues
                first (its parts share one psum tile)."""
                STEP_BUDGET = 620.0
                n_steps = 16 * NH * 2
                scores(0)
                scores(1)
                pending = sorted(work, key=lambda w: (w["due"], w["release"]))
                current = None
                for step in range(n_steps):
                    h, qh, j = phase_of(step)
                    ptile = ppool.tile(
                        [P, 2, 512], dt.bfloat16, tag="p", name=f"p{step}"
                    )
                    nc.scalar.activation(
                        ptile.rearrange("p a b -> p (a b)"),
                        rings[step % 2].rearrange("p a b -> p (a b)"),
                        mybir.ActivationFunctionType.Exp,
                        scale=SCALE / 2.0,
                    )
                    if step + 2 < n_steps:
                        scores(step + 2)
                    for qt in range(8):
                        nc.tensor.matmul(
                            pv[:, qt, 0 : HD + 1],
                            ptile[:, qt // 4, (qt % 4) * P : (qt % 4 + 1) * P],
                            va[:, j, h, :],
                            start=(j == 0 and qt % 4 == 0),
                            stop=(j == ST - 1),
                            skip_group_check=True,
                        )
                    budget = STEP_BUDGET
                    while budget > 0:
                        if current is None:
                            cand = [w for w in pending if w["release"] <= step]
                            if not cand:
                                break
                            current = cand[0]
                            pending.remove(current)
                            assert current["due"] >= step, (
                                f"work item overdue: emitted step {step}, "
                                f"due {current['due']}"
                            )
                        fn, cost = current["fns"].pop(0)
                        fn()
                        budget -= cost
                        if not current["fns"]:
                            current = None
                    if j == ST - 1:
                        emit_evac(h, qh)
                # anything left (tail work: release >= n_steps)
                leftovers = ([current] if current else []) + pending
                leftovers.sort(key=lambda w: (w["release"], w["due"]))
                for w in leftovers:
                    for fn, _ in w["fns"]:
                        fn()

            # ---------------- emission schedule -------------------------
            # Pre-attention prefix: K/Q d-tile 0 for the first half
            # accumulate in the four (still idle) ring slots in parallel,
            # e-interleaved so each MM fires as its x^T e-tile DMA lands.
            pref = [
                ("k", 0, rings[0], 0), ("q", 0, rings[0], 1),
                ("k", 1, rings[1], 0), ("q", 1, rings[1], 1),
            ]
            for e in range(ET):
                for which, c, rg, slot in pref:
                    w_ = wk if which == "k" else wq
                    nc.tensor.matmul(
                        rg[:, slot, :],
                        w_[:, e, 0:P],
                        xt[:, e, c * 512 : (c + 1) * 512],
                        start=(e == 0),
                        stop=(e == ET - 1),
                    )
            for which, c, rg, slot in pref:
                b_, dst = (bk, kt8) if which == "k" else (bq, qt8)
                nc.vector.tensor_scalar_add(
                    dst[:, 0, c * 512 : (c + 1) * 512], rg[:, slot, :], b_[:, 0:1]
                )
            for st in range(6):
                v_chunk(0, st)()

            # Work items with release/due steps.  Due dates: a K/Q chunk
            # feeding scores(x) must finish by step x-3 (scores run two
            # steps ahead and precede thunks within a step); a V chunk
            # feeding PV(x) by step x-1.
            KQC, VC, TRC, OPC = 950.0, 340.0, 250.0, 650.0
            work = []

            def add(release, due, fns, cost):
                work.append(
                    {"release": release, "due": due,
                     "fns": [(f, cost) for f in fns]}
                )

            # V: head 0 tiles 6..15 (0..5 in the prefix), then all other heads
            for st in range(6, ST):
                add(0, st - 1, [v_chunk(0, st)], VC)
            for h in range(1, NH):
                for st in range(ST):
                    add(0, 16 * h + st - 1, [v_chunk(h, st)], VC)
            # K: d-tile 0 chunks 2/3 (0/1 in the prefix), d-tiles 1..3 all
            for c in (2, 3):
                add(0, 4 * c - 3, kq_halves("k", 0, c), KQC / 2)
            for t2 in range(1, DT):
                for c in range(4):
                    add(0, 32 * t2 + 4 * c - 3, kq_halves("k", t2, c), KQC / 2)
            # Q: low half (chunks 0/1) due at (2t, qh0); high half at qh1
            for t2 in range(1, DT):
                for c in (0, 1):
                    add(0, 32 * t2 - 3, kq_halves("q", t2, c), KQC / 2)
            for t2 in range(DT):
                for c in (2, 3):
                    add(0, 128 + 32 * t2 - 3, kq_halves("q", t2, c), KQC / 2)
            # transposes: tight due dates so they spread right after their
            # pair completes (the single transpose buffer serializes
            # clumped transposes at ~450ns each)
            for t2 in range(DT):
                rel = 32 * t2 + 32
                add(rel, rel + 28 if t2 < DT - 1 else 145,
                    [transpose_item(t2, 0, qt) for qt in range(8)], TRC)
            for t2 in range(DT):
                rel = 176 + 32 * t2
                add(rel, rel + 28 if t2 < DT - 1 else 10**6,
                    [transpose_item(t2, 1, qt) for qt in range(8)], TRC)
            # outproj quarters 0/1 during half 1; quarters 2/3 in the tail
            for eo in range(ET):
                for qq in (0, 1):
                    add(146, 254, outproj_halves(eo, qq), OPC)
            # tail outproj uses the (then idle) scores ring as rotating psum
            ci = [0]

            def tail_outproj(eo, qq):
                def go():
                    ps = rings[ci[0] % 2][:, (ci[0] // 2) % 2, :]
                    ci[0] += 1
                    q0 = qq * 512
                    for t in range(DT):
                        nc.tensor.matmul(
                            ps,
                            wo[:, t, eo * P : (eo + 1) * P],
                            scb[:, t, q0 : q0 + 512],
                            start=(t == 0),
                            stop=(t == DT - 1),
                        )
                    ot = outpool.tile(
                        [P, 512], dt.float32, tag="ot", name=f"ot{eo}{qq}"
                    )
                    nc.vector.tensor_scalar_add(ot[:], ps, bo[:, eo : eo + 1])
                    nc.sync.dma_start(
                        out_d[eo * P : (eo + 1) * P, q0 : q0 + 512], ot[:]
                    )
                return go

            for eo in range(ET):
                for qq in (2, 3):
                    add(10**6, 10**6, [tail_outproj(eo, qq)], OPC)

            # Run the whole attention stream (tail work included).
            run_stream(work)

    nc.compile()
    return nc


def _prep_inputs(x, W_qkv, b_qkv, W_out, b_out):
    """Host-side sharding + layout prep. Returns per-core input maps."""
    w = W_qkv.reshape(E, H, 3, HD)
    b3 = b_qkv.reshape(H, 3, HD)
    iden = np.eye(P, dtype=np.float32).astype(_BF16)

    in_maps = []
    for core in range(N_CORES):
        b, hg = core // 2, core % 2
        hs = slice(hg * NH, (hg + 1) * NH)
        xt = np.ascontiguousarray(x[b].T).astype(_BF16)           # [E, S]
        wq = np.ascontiguousarray(w[:, hs, 0, :].reshape(E, 512)).astype(_BF16)
        wk = np.ascontiguousarray(w[:, hs, 1, :].reshape(E, 512)).astype(_BF16)
        wv = np.ascontiguousarray(w[:, hs, 2, :].reshape(E, 512)).astype(_BF16)
        wo = np.ascontiguousarray(W_out[hg * 512 : (hg + 1) * 512, :]).astype(_BF16)
        bq = np.ascontiguousarray(b3[hs, 0, :].reshape(DT, P).T).astype(np.float32)
        bk = np.ascontiguousarray(b3[hs, 1, :].reshape(DT, P).T).astype(np.float32)
        bv = np.ascontiguousarray(b3[hs, 2, :].reshape(1, 512)).astype(_BF16)
        bo = (np.ascontiguousarray(b_out.reshape(ET, P).T) * (1.0 if hg == 0 else 0.0)).astype(np.float32)
        in_maps.append(
            {
                "xt": xt,
                "wq": wq,
                "wk": wk,
                "wv": wv,
                "wo": wo,
                "bq": bq,
                "bk": bk,
                "bv": bv,
                "bo": bo,
                "iden": iden,
            }
        )
    return in_maps


def run_raw(x, W_qkv, b_qkv, W_out, b_out, trace=False, **kw):
    """Run on hardware; returns (full_output [B,S,E] f32, BassKernelResults)."""
    global _cached
    from concourse.bass_utils import run_bass_kernel_spmd

    if _cached is None:
        _cached = _build()
    nc = _cached
    in_maps = _prep_inputs(
        np.asarray(x), np.asarray(W_qkv), np.asarray(b_qkv),
        np.asarray(W_out), np.asarray(b_out),
    )
    res = run_bass_kernel_spmd(
        nc, in_maps, core_ids=list(range(N_CORES)), trace=trace, **kw
    )
    out = np.empty((B, S, E), dtype=np.float32)
    for b in range(B):
        acc = np.asarray(res.results[2 * b]["out"]) + np.asarray(
            res.results[2 * b + 1]["out"]
        )
        out[b] = acc.T
    return out, res


def kernel(x, W_qkv, b_qkv, W_out, b_out):
    out, _ = run_raw(x, W_qkv, b_qkv, W_out, b_out, trace=False)
    return out


# revision 37
# speedup vs baseline: 1.4882x; 1.0362x over previous
"""Multi-head attention (B=4, S=2048, E=1024, H=16) on 8 TRN2 NeuronCores.

Sharding: batch x head-group tensor parallel -- core c = 2*b + hg handles
batch b and heads hg*8 .. hg*8+7 for ALL 2048 queries.  Q/K/V projections
are column-split by head (each core projects only its 8 heads); the output
projection is row-split (each core contracts its 512 E-rows of W_out) and
produces a partial [E, S] output that the HOST sums across the core pair
while unsharding (the "all-reduce" of the sharding hint, done on host).

Per-core kernel:
  - Q^T/K^T projections (bf16 matmul, fp32 PSUM) evacuated with fused
    bias-add + fp8e4 quantization (DVE tensor_scalar_add, fp8 out).
  - scores via fp8 DoubleRow matmuls: contraction d=64 fed as
    [64 part, 2(dup, stride 0), N]; the duplicated group doubles the
    result and the exp activation scale absorbs the factor 2.
    Cost: 0.5 cycles/row (vs 1.0 bf16).
  - exp on ScalarE (the bottleneck engine, ~266us busy): one [128, 1024]
    activation per key-tile j, reading two adjacent 512-wide slots of a
    manual 4-slot PSUM ring (slots 2j%4, 2j%4+1 -> flat AP; the two slot
    pairs double-buffer).  Scores for j+1 are emitted BEFORE the PV
    matmuls of j so the exp stream never waits on PE's in-order queue.
  - PV in the FLIPPED orientation: out[q=128, 65] = P_tile.T @ [V | ones]
    (all 128 output partitions vs 65 the naive way); the ones column is
    the softmax denominator per query row.  The 8 per-qt accumulators
    live in one [128, 8, 128] tile (qt stride 512B -> no bank crossing);
    PSUM start=True zero-fills a whole 2KB region, so only the first
    matmul touching each bank uses start=True and the rest rely on the
    pending-zero overwrite semantics (no memset needed).
  - normalization: per-partition reciprocal of the denominator column +
    tensor_scalar multiply -> O in [q, d]; PE-transpose (identity
    matmul) back to O^T for the out projection.

Schedule: 2 query-half phase groups x 8 heads x 16 key tiles (j).
Per j-step the PE also runs one or two small interleaved chunks: V
projection for the NEXT head (just-in-time, ~240ns each), K/Q
projection half-chunks (~850ns), O^T transposes, and the first half's
output projection (during the second half).  Only the second half's
output projection is a serial tail.
"""

import sys

if "/opt/trn_rl_repo" not in sys.path:
    sys.path.insert(0, "/opt/trn_rl_repo")

import numpy as np
import ml_dtypes

B, S, E, H = 4, 2048, 1024, 16
P = 128
HD = 64           # head dim
NH = 8            # heads per core
DT = 4            # d-tiles (head pairs) per core
ET = E // P       # 8 e-tiles (contraction for projections)
ST = S // P       # 16 key tiles
N_CORES = 8
QH = S // 2       # query half width (1024)
SCALE = 1.0 / float(np.sqrt(HD))

_BF16 = ml_dtypes.bfloat16

_cached = None


def _build():
    import concourse.bass as bass
    import concourse.tile as tile
    import concourse.mybir as mybir
    from concourse import bacc

    dt = mybir.dt
    nc = bacc.Bacc("TRN2", target_bir_lowering=False, debug=False)

    xt_d = nc.dram_tensor("xt", [E, S], dt.bfloat16, kind="ExternalInput").ap()
    wq_d = nc.dram_tensor("wq", [E, 512], dt.bfloat16, kind="ExternalInput").ap()
    wk_d = nc.dram_tensor("wk", [E, 512], dt.bfloat16, kind="ExternalInput").ap()
    wv_d = nc.dram_tensor("wv", [E, 512], dt.bfloat16, kind="ExternalInput").ap()
    wo_d = nc.dram_tensor("wo", [512, E], dt.bfloat16, kind="ExternalInput").ap()
    bq_d = nc.dram_tensor("bq", [P, DT], dt.float32, kind="ExternalInput").ap()
    bk_d = nc.dram_tensor("bk", [P, DT], dt.float32, kind="ExternalInput").ap()
    bv_d = nc.dram_tensor("bv", [1, 512], dt.bfloat16, kind="ExternalInput").ap()
    bo_d = nc.dram_tensor("bo", [P, ET], dt.float32, kind="ExternalInput").ap()
    iden_d = nc.dram_tensor("iden", [P, P], dt.bfloat16, kind="ExternalInput").ap()
    out_d = nc.dram_tensor("out", [E, S], dt.bfloat16, kind="ExternalOutput").ap()

    DR = mybir.MatmulPerfMode.DoubleRow

    with tile.TileContext(nc) as tc:
        with (
            tc.tile_pool(name="const", bufs=1) as cpool,
            tc.tile_pool(name="acts", bufs=1) as apool,
            tc.tile_pool(name="pp", bufs=3) as ppool,        # P (exp out)
            tc.tile_pool(name="oqp", bufs=2) as oqpool,      # O [q, dd] staging
            tc.tile_pool(name="recp", bufs=2) as recpool,    # reciprocals
            tc.tile_pool(name="outs", bufs=8) as outpool,    # out staging
            tc.tile_pool(name="pssc", bufs=2, space="PSUM") as scpool,   # 4 banks
            tc.tile_pool(name="pspv", bufs=1, space="PSUM") as pvpool,   # 2 banks
            tc.tile_pool(name="pspj", bufs=1, space="PSUM") as pjpool,   # 1 bank
            tc.tile_pool(name="pstp", bufs=1, space="PSUM") as tppool,   # 1 bank
        ):
            # ---------------- constants / inputs -----------------------
            xt = cpool.tile([P, ET, S], dt.bfloat16)
            wq = cpool.tile([P, ET, 512], dt.bfloat16)
            wk = cpool.tile([P, ET, 512], dt.bfloat16)
            wv = cpool.tile([P, ET, 512], dt.bfloat16)
            wo = cpool.tile([P, DT, E], dt.bfloat16)
            bq = cpool.tile([P, DT], dt.float32)
            bk = cpool.tile([P, DT], dt.float32)
            bv = cpool.tile([1, 512], dt.bfloat16)
            bo = cpool.tile([P, ET], dt.float32)
            iden = cpool.tile([P, P], dt.bfloat16)
            ones1 = cpool.tile([1, P], dt.bfloat16)

            # activations
            qt8 = apool.tile([P, DT, S], dt.float8e4)   # Q^T (bias+fp8)
            kt8 = apool.tile([P, DT, S], dt.float8e4)   # K^T (bias+fp8)
            va = apool.tile([P, ST, NH, HD + 1], dt.bfloat16)  # V | ones
            scb = apool.tile([P, DT, S], dt.bfloat16)   # O^T (normalized)

            # Long-lived PSUM tiles.  Dependency tracking is TILE-granular,
            # so the scores ring is TWO alternating tiles: exp(step) then
            # only depends on its own tile's scores, and scores(step+2)
            # (same tile) WAR-waits exp(step) -- the other tile streams
            # freely underneath.
            rings = [
                scpool.tile([P, 2, 512], dt.float32, tag="sc", name="ringA"),
                scpool.tile([P, 2, 512], dt.float32, tag="sc", name="ringB"),
            ]
            pv = pvpool.tile([P, 8, P], dt.float32, tag="pv", name="pv")

            # DMA transfers serialize on the global DMA device, so order by
            # criticality: d-tile-0 K/Q slices and head-0 V cols first, the
            # full x^T (bandwidth floor ~12.6us), then everything else.
            wkr = wk_d.rearrange("(eo p) c -> p eo c", p=P)
            wqr = wq_d.rearrange("(eo p) c -> p eo c", p=P)
            wvr = wv_d.rearrange("(eo p) c -> p eo c", p=P)
            nc.sync.dma_start(wk[:, :, 0:P], wkr[:, :, 0:P])
            nc.sync.dma_start(wq[:, :, 0:P], wqr[:, :, 0:P])
            for e in range(ET):
                nc.sync.dma_start(xt[:, e, :], xt_d[e * P : (e + 1) * P, :])
            nc.sync.dma_start(bk[:], bk_d)
            nc.sync.dma_start(bq[:], bq_d)
            nc.sync.dma_start(wv[:, :, 0:HD], wvr[:, :, 0:HD])
            nc.sync.dma_start(bv[:], bv_d)
            nc.sync.dma_start(wv[:, :, HD:512], wvr[:, :, HD:512])
            nc.sync.dma_start(wk[:, :, P:512], wkr[:, :, P:512])
            nc.sync.dma_start(wq[:, :, P:512], wqr[:, :, P:512])
            nc.sync.dma_start(iden[:], iden_d)
            nc.sync.dma_start(wo[:], wo_d.rearrange("(eo p) c -> p eo c", p=P))
            nc.sync.dma_start(bo[:], bo_d)
            nc.gpsimd.memset(ones1[:], 1.0)
            nc.gpsimd.memset(va[:, :, :, HD : HD + 1], 1.0)

            # ---------------- small-chunk emitters ----------------------
            # kq/outproj chunks come as (partA, partB) sharing one psum tile;
            # with a single-buffer proj pool they are also adjacent-safe.

            def kq_halves(which, t, c):
                w_, b_, dst = (wk, bk, kt8) if which == "k" else (wq, bq, qt8)
                holder = {}

                def part0():
                    ps = pjpool.tile(
                        [P, 512], dt.float32, tag="pj", name=f"{which}{t}{c}"
                    )
                    holder[0] = ps
                    for e in range(4):
                        nc.tensor.matmul(
                            ps[:],
                            w_[:, e, t * P : (t + 1) * P],
                            xt[:, e, c * 512 : (c + 1) * 512],
                            start=(e == 0),
                            stop=False,
                        )

                def part1():
                    ps = holder[0]
                    for e in range(4, ET):
                        nc.tensor.matmul(
                            ps[:],
                            w_[:, e, t * P : (t + 1) * P],
                            xt[:, e, c * 512 : (c + 1) * 512],
                            start=False,
                            stop=(e == ET - 1),
                        )
                    nc.vector.tensor_scalar_add(
                        dst[:, t, c * 512 : (c + 1) * 512], ps[:], b_[:, t : t + 1]
                    )

                return [part0, part1]

            def v_chunk(h, st):
                """V rows for (head h, key tile st): [128 keys, 64] + bias.
                Uses the transpose-psum bank (tiny tiles) to stay off the
                kq/outproj pipeline."""
                def go():
                    ps = tppool.tile([P, P], dt.float32, tag="vps", name=f"v{h}{st}")
                    for e in range(ET):
                        nc.tensor.matmul(
                            ps[:, 0:HD],
                            xt[:, e, st * P : (st + 1) * P],
                            wv[:, e, h * HD : (h + 1) * HD],
                            start=(e == 0),
                            stop=False,
                        )
                    nc.tensor.matmul(
                        ps[:, 0:HD],
                        ones1[0:1, :],
                        bv[0:1, h * HD : (h + 1) * HD],
                        start=False,
                        stop=True,
                    )
                    nc.vector.tensor_copy(va[:, st, h, 0:HD], ps[:, 0:HD])
                return go

            def outproj_halves(eo, qq):
                """Partial out^T tile [128 Eo, 512 q] for query quarter qq."""
                holder = {}
                q0 = qq * 512

                def part0():
                    ps = pjpool.tile([P, 512], dt.float32, tag="pj", name=f"o{eo}{qq}")
                    holder[0] = ps
                    for t in (0, 1):
                        nc.tensor.matmul(
                            ps[:],
                            wo[:, t, eo * P : (eo + 1) * P],
                            scb[:, t, q0 : q0 + 512],
                            start=(t == 0),
                            stop=False,
                        )

                def part1():
                    ps = holder[0]
                    for t in (2, 3):
                        nc.tensor.matmul(
                            ps[:],
                            wo[:, t, eo * P : (eo + 1) * P],
                            scb[:, t, q0 : q0 + 512],
                            start=False,
                            stop=(t == DT - 1),
                        )
                    ot = outpool.tile(
                        [P, 512], dt.bfloat16, tag="ot", name=f"oe{eo}{qq}"
                    )
                    nc.vector.tensor_scalar_add(ot[:], ps[:], bo[:, eo : eo + 1])
                    nc.sync.dma_start(
                        out_d[eo * P : (eo + 1) * P, q0 : q0 + 512], ot[:]
                    )

                return [part0, part1]

            oq_tiles = {}

            def transpose_item(t, qh, qt):
                """oq [q, dd of pair t] -> scb[:, t, ...] via PE transpose.
                Uses the same psum bank as v_chunk (different tag would
                double-book the bank, so share tag/shape via bitcast)."""
                def go():
                    tp = tppool.tile([P, P], dt.float32, tag="vps", name=f"tp{t}{qh}{qt}")
                    tpb = tp[:, 0:HD].bitcast(dt.bfloat16)
                    nc.tensor.transpose(tpb, oq_tiles[(t, qh)][:, qt, :], iden[:])
                    q0 = qh * QH + qt * P
                    nc.vector.tensor_copy(scb[:, t, q0 : q0 + P], tpb)
                return go

            def transpose_batch(t, qh):
                """Tail variant: all 8 transposes into the idle pv banks,
                then the 8 copies -- avoids the 1-buffer ping-pong."""
                def go():
                    views = []
                    for qt in range(8):
                        v = pv[:, qt, 0:HD].bitcast(dt.bfloat16)
                        nc.tensor.transpose(v, oq_tiles[(t, qh)][:, qt, :], iden[:])
                        views.append(v)
                    q0 = qh * QH
                    for qt in range(8):
                        nc.vector.tensor_copy(
                            scb[:, t, q0 + qt * P : q0 + (qt + 1) * P], views[qt]
                        )
                return go

            # ---------------- attention stream ---------------------------
            # One global stream of 256 j-steps (16 phases x 16 key tiles).
            # Per step: exp(step) on ACT, then on PE scores(step+2) (its ring
            # slots were just freed by exp(step)), then the PV wave of step,
            # then interleaved thunks.  This keeps the serial chain between
            # consecutive exps down to one sem hop, across phase boundaries
            # included.
            def scores(step):
                h, qh, j = phase_of(step)
                t, hp = h // 2, (h % 2) * HD
                q0 = qh * QH
                rg = rings[step % 2]
                for qc in range(2):
                    nc.tensor.matmul(
                        rg[:, qc, :],
                        kt8[hp : hp + HD, t, j * P : (j + 1) * P]
                        .unsqueeze(1)
                        .broadcast_to((HD, 2, P)),
                        qt8[hp : hp + HD, t, q0 + qc * 512 : q0 + (qc + 1) * 512]
                        .unsqueeze(1)
                        .broadcast_to((HD, 2, 512)),
                        start=True,
                        stop=True,
                        perf_mode=DR,
                    )

            def phase_of(step):
                phase, j = divmod(step, ST)
                qh, h = divmod(phase, NH)
                return h, qh, j

            def emit_evac(h, qh):
                t, half = h // 2, h % 2
                if half == 0:
                    oq_tiles[(t, qh)] = oqpool.tile(
                        [P, 8, P], dt.bfloat16, tag="oq", name=f"oq{t}{qh}"
                    )
                oq = oq_tiles[(t, qh)]
                # one fast copy releases the pv tile for the next phase's
                # matmuls; normalize off the copy (off the critical path)
                pvc = recpool.tile([P, 8, HD + 1], dt.float32, tag="pvc", name=f"pc{h}{qh}")
                nc.vector.tensor_copy(pvc[:], pv[:, :, 0 : HD + 1])
                rec = recpool.tile([P, 8], dt.float32, tag="rec", name=f"rc{h}{qh}")
                scr = recpool.tile([P, 8], dt.float32, tag="scr", name=f"sr{h}{qh}")
                nc.vector.reciprocal_approx_accurate(
                    rec[:], pvc[:, :, HD : HD + 1].rearrange("p a b -> p (a b)"), scr[:]
                )
                for qt in range(8):
                    nc.vector.tensor_scalar(
                        oq[:, qt, half * HD : (half + 1) * HD],
                        pvc[:, qt, 0:HD],
                        rec[:, qt : qt + 1],
                        None,
                        op0=mybir.AluOpType.mult,
                    )

            def run_stream(work):
                """work: list of dicts {release, due, fns: [(fn, cost), ...]}.
                Per step, spend ~STEP_BUDGET ns of PE time on the earliest-due
                released items; a partially-emitted item always continues
                first (its parts share one psum tile)."""
                STEP_BUDGET = 620.0
                n_steps = 16 * NH * 2
                scores(0)
                scores(1)
                pending = sorted(work, key=lambda w: (w["due"], w["release"]))
                current = None
                for step in range(n_steps):
                    h, qh, j = phase_of(step)
                    ptile = ppool.tile(
                        [P, 2, 512], dt.bfloat16, tag="p", name=f"p{step}"
                    )
                    nc.scalar.activation(
                        ptile.rearrange("p a b -> p (a b)"),
                        rings[step % 2].rearrange("p a b -> p (a b)"),
                        mybir.ActivationFunctionType.Exp,
                        scale=SCALE / 2.0,
                    )
                    if step + 2 < n_steps:
                        scores(step + 2)
                    for qt in range(8):
                        nc.tensor.matmul(
                            pv[:, qt, 0 : HD + 1],
                            ptile[:, qt // 4, (qt % 4) * P : (qt % 4 + 1) * P],
                            va[:, j, h, :],
                            start=(j == 0 and qt % 4 == 0),
                            stop=(j == ST - 1),
                            skip_group_check=True,
                        )
                    budget = STEP_BUDGET
                    while budget > 0:
                        if current is None:
                            cand = [w for w in pending if w["release"] <= step]
                            if not cand:
                                break
                            current = cand[0]
                            pending.remove(current)
                            assert current["due"] >= step, (
                                f"work item overdue: emitted step {step}, "
                                f"due {current['due']}"
                            )
                        fn, cost = current["fns"].pop(0)
                        fn()
                        budget -= cost
                        if not current["fns"]:
                            current = None
                    if j == ST - 1:
                        emit_evac(h, qh)
                # anything left (tail work: release >= n_steps)
                leftovers = ([current] if current else []) + pending
                leftovers.sort(key=lambda w: (w["release"], w["due"]))
                for w in leftovers:
                    for fn, _ in w["fns"]:
                        fn()

            # ---------------- emission schedule -------------------------
            # Pre-attention prefix: K/Q d-tile 0 for the first half
            # accumulate in the four (still idle) ring slots in parallel,
            # e-interleaved so each MM fires as its x^T e-tile DMA lands.
            pref = [
                ("k", 0, rings[0], 0), ("q", 0, rings[0], 1),
                ("k", 1, rings[1], 0), ("q", 1, rings[1], 1),
            ]
            for e in range(ET):
                for which, c, rg, slot in pref:
                    w_ = wk if which == "k" else wq
                    nc.tensor.matmul(
                        rg[:, slot, :],
                        w_[:, e, 0:P],
                        xt[:, e, c * 512 : (c + 1) * 512],
                        start=(e == 0),
                        stop=(e == ET - 1),
                    )
            for which, c, rg, slot in (pref[0], pref[1], pref[3], pref[2]):
                b_, dst = (bk, kt8) if which == "k" else (bq, qt8)
                nc.vector.tensor_scalar_add(
                    dst[:, 0, c * 512 : (c + 1) * 512], rg[:, slot, :], b_[:, 0:1]
                )
            for st in range(ST):
                v_chunk(0, st)()

            # Work items with release/due steps.  Due dates: a K/Q chunk
            # feeding scores(x) must finish by step x-3 (scores run two
            # steps ahead and precede thunks within a step); a V chunk
            # feeding PV(x) by step x-1.
            KQC, VC, TRC, OPC = 950.0, 340.0, 250.0, 650.0
            work = []

            def add(release, due, fns, cost):
                work.append(
                    {"release": release, "due": due,
                     "fns": [(f, cost) for f in fns]}
                )

            # V: head 0 entirely in the prefix; other heads here
            for h in range(1, NH):
                for st in range(ST):
                    add(0, 16 * h + st - 1, [v_chunk(h, st)], VC)
            # K: d-tile 0 chunks 2/3 (0/1 in the prefix), d-tiles 1..3 all
            for c in (2, 3):
                add(0, 4 * c - 3, kq_halves("k", 0, c), KQC / 2)
            for t2 in range(1, DT):
                for c in range(4):
                    add(0, 32 * t2 + 4 * c - 3, kq_halves("k", t2, c), KQC / 2)
            # Q: low half (chunks 0/1) due at (2t, qh0); high half at qh1
            for t2 in range(1, DT):
                for c in (0, 1):
                    add(0, 32 * t2 - 3, kq_halves("q", t2, c), KQC / 2)
            for t2 in range(DT):
                for c in (2, 3):
                    add(0, 128 + 32 * t2 - 3, kq_halves("q", t2, c), KQC / 2)
            # transposes: tight due dates so they spread right after their
            # pair completes (the single transpose buffer serializes
            # clumped transposes at ~450ns each)
            for t2 in range(DT):
                rel = 32 * t2 + 32
                add(rel, rel + 28 if t2 < DT - 1 else 145,
                    [transpose_item(t2, 0, qt) for qt in range(8)], TRC)
            for t2 in range(DT - 1):
                rel = 176 + 32 * t2
                add(rel, rel + 28,
                    [transpose_item(t2, 1, qt) for qt in range(8)], TRC)
            add(10**6 - 1, 10**6, [transpose_batch(DT - 1, 1)], TRC)
            # outproj quarters 0/1 during half 1; quarters 2/3 in the tail
            for eo in range(ET):
                for qq in (0, 1):
                    add(146, 254, outproj_halves(eo, qq), OPC)
            # tail outproj uses the (then idle) scores ring as rotating psum
            ci = [0]

            def tail_outproj(eo, qq):
                def go():
                    ps = rings[ci[0] % 2][:, (ci[0] // 2) % 2, :]
                    ci[0] += 1
                    q0 = qq * 512
                    for t in range(DT):
                        nc.tensor.matmul(
                            ps,
                            wo[:, t, eo * P : (eo + 1) * P],
                            scb[:, t, q0 : q0 + 512],
                            start=(t == 0),
                            stop=(t == DT - 1),
                        )
                    ot = outpool.tile(
                        [P, 512], dt.bfloat16, tag="ot", name=f"ot{eo}{qq}"
                    )
                    nc.vector.tensor_scalar_add(ot[:], ps, bo[:, eo : eo + 1])
                    nc.sync.dma_start(
                        out_d[eo * P : (eo + 1) * P, q0 : q0 + 512], ot[:]
                    )
                return go

            for eo in range(ET):
                for qq in (2, 3):
                    add(10**6, 10**6, [tail_outproj(eo, qq)], OPC)

            # Run the whole attention stream (tail work included).
            run_stream(work)

    nc.compile()
    return nc


def _prep_inputs(x, W_qkv, b_qkv, W_out, b_out):
    """Host-side sharding + layout prep. Returns per-core input maps."""
    w = W_qkv.reshape(E, H, 3, HD)
    b3 = b_qkv.reshape(H, 3, HD)
    iden = np.eye(P, dtype=np.float32).astype(_BF16)

    in_maps = []
    for core in range(N_CORES):
        b, hg = core // 2, core % 2
        hs = slice(hg * NH, (hg + 1) * NH)
        xt = np.ascontiguousarray(x[b].T).astype(_BF16)           # [E, S]
        wq = np.ascontiguousarray(w[:, hs, 0, :].reshape(E, 512)).astype(_BF16)
        wk = np.ascontiguousarray(w[:, hs, 1, :].reshape(E, 512)).astype(_BF16)
        wv = np.ascontiguousarray(w[:, hs, 2, :].reshape(E, 512)).astype(_BF16)
        wo = np.ascontiguousarray(W_out[hg * 512 : (hg + 1) * 512, :]).astype(_BF16)
        bq = np.ascontiguousarray(b3[hs, 0, :].reshape(DT, P).T).astype(np.float32)
        bk = np.ascontiguousarray(b3[hs, 1, :].reshape(DT, P).T).astype(np.float32)
        bv = np.ascontiguousarray(b3[hs, 2, :].reshape(1, 512)).astype(_BF16)
        bo = (np.ascontiguousarray(b_out.reshape(ET, P).T) * (1.0 if hg == 0 else 0.0)).astype(np.float32)
        in_maps.append(
            {
                "xt": xt,
                "wq": wq,
                "wk": wk,
                "wv": wv,
                "wo": wo,
                "bq": bq,
                "bk": bk,
                "bv": bv,
                "bo": bo,
                "iden": iden,
            }
        )
    return in_maps


def run_raw(x, W_qkv, b_qkv, W_out, b_out, trace=False, **kw):
    """Run on hardware; returns (full_output [B,S,E] f32, BassKernelResults)."""
    global _cached
    from concourse.bass_utils import run_bass_kernel_spmd

    if _cached is None:
        _cached = _build()
    nc = _cached
    in_maps = _prep_inputs(
        np.asarray(x), np.asarray(W_qkv), np.asarray(b_qkv),
        np.asarray(W_out), np.asarray(b_out),
    )
    res = run_bass_kernel_spmd(
        nc, in_maps, core_ids=list(range(N_CORES)), trace=trace, **kw
    )
    out = np.empty((B, S, E), dtype=np.float32)
    for b in range(B):
        acc = np.asarray(res.results[2 * b]["out"]).astype(np.float32) + np.asarray(
            res.results[2 * b + 1]["out"]
        ).astype(np.float32)
        out[b] = acc.T
    return out, res


def kernel(x, W_qkv, b_qkv, W_out, b_out):
    out, _ = run_raw(x, W_qkv, b_qkv, W_out, b_out, trace=False)
    return out


# revision 38
# speedup vs baseline: 1.5035x; 1.0103x over previous
"""Multi-head attention (B=4, S=2048, E=1024, H=16) on 8 TRN2 NeuronCores.

Sharding: batch x head-group tensor parallel -- core c = 2*b + hg handles
batch b and heads hg*8 .. hg*8+7 for ALL 2048 queries.  Q/K/V projections
are column-split by head (each core projects only its 8 heads); the output
projection is row-split (each core contracts its 512 E-rows of W_out) and
produces a partial [E, S] output that the HOST sums across the core pair
while unsharding (the "all-reduce" of the sharding hint, done on host).

Per-core kernel:
  - Q^T/K^T projections (bf16 matmul, fp32 PSUM) evacuated with fused
    bias-add + fp8e4 quantization (DVE tensor_scalar_add, fp8 out).
  - scores via fp8 DoubleRow matmuls: contraction d=64 fed as
    [64 part, 2(dup, stride 0), N]; the duplicated group doubles the
    result and the exp activation scale absorbs the factor 2.
    Cost: 0.5 cycles/row (vs 1.0 bf16).
  - exp on ScalarE (the bottleneck engine, ~266us busy): one [128, 1024]
    activation per key-tile j, reading two adjacent 512-wide slots of a
    manual 4-slot PSUM ring (slots 2j%4, 2j%4+1 -> flat AP; the two slot
    pairs double-buffer).  Scores for j+1 are emitted BEFORE the PV
    matmuls of j so the exp stream never waits on PE's in-order queue.
  - PV in the FLIPPED orientation: out[q=128, 65] = P_tile.T @ [V | ones]
    (all 128 output partitions vs 65 the naive way); the ones column is
    the softmax denominator per query row.  The 8 per-qt accumulators
    live in one [128, 8, 128] tile (qt stride 512B -> no bank crossing);
    PSUM start=True zero-fills a whole 2KB region, so only the first
    matmul touching each bank uses start=True and the rest rely on the
    pending-zero overwrite semantics (no memset needed).
  - normalization: per-partition reciprocal of the denominator column +
    tensor_scalar multiply -> O in [q, d]; PE-transpose (identity
    matmul) back to O^T for the out projection.

Schedule: 2 query-half phase groups x 8 heads x 16 key tiles (j).
Per j-step the PE also runs one or two small interleaved chunks: V
projection for the NEXT head (just-in-time, ~240ns each), K/Q
projection half-chunks (~850ns), O^T transposes, and the first half's
output projection (during the second half).  Only the second half's
output projection is a serial tail.
"""

import sys

if "/opt/trn_rl_repo" not in sys.path:
    sys.path.insert(0, "/opt/trn_rl_repo")

import numpy as np
import ml_dtypes

B, S, E, H = 4, 2048, 1024, 16
P = 128
HD = 64           # head dim
NH = 8            # heads per core
DT = 4            # d-tiles (head pairs) per core
ET = E // P       # 8 e-tiles (contraction for projections)
ST = S // P       # 16 key tiles
N_CORES = 8
QH = S // 2       # query half width (1024)
SCALE = 1.0 / float(np.sqrt(HD))

_BF16 = ml_dtypes.bfloat16

_cached = None


def _build():
    import concourse.bass as bass
    import concourse.tile as tile
    import concourse.mybir as mybir
    from concourse import bacc

    dt = mybir.dt
    nc = bacc.Bacc("TRN2", target_bir_lowering=False, debug=False)

    xt_d = nc.dram_tensor("xt", [E, S], dt.bfloat16, kind="ExternalInput").ap()
    wq_d = nc.dram_tensor("wq", [E, 512], dt.bfloat16, kind="ExternalInput").ap()
    wk_d = nc.dram_tensor("wk", [E, 512], dt.bfloat16, kind="ExternalInput").ap()
    wv_d = nc.dram_tensor("wv", [E, 512], dt.bfloat16, kind="ExternalInput").ap()
    wo_d = nc.dram_tensor("wo", [512, E], dt.bfloat16, kind="ExternalInput").ap()
    bq_d = nc.dram_tensor("bq", [P, DT], dt.float32, kind="ExternalInput").ap()
    bk_d = nc.dram_tensor("bk", [P, DT], dt.float32, kind="ExternalInput").ap()
    bv_d = nc.dram_tensor("bv", [1, 512], dt.bfloat16, kind="ExternalInput").ap()
    bo_d = nc.dram_tensor("bo", [P, ET], dt.float32, kind="ExternalInput").ap()
    iden_d = nc.dram_tensor("iden", [P, P], dt.bfloat16, kind="ExternalInput").ap()
    out_d = nc.dram_tensor("out", [E, S], dt.bfloat16, kind="ExternalOutput").ap()

    DR = mybir.MatmulPerfMode.DoubleRow

    with tile.TileContext(nc) as tc:
        with (
            tc.tile_pool(name="const", bufs=1) as cpool,
            tc.tile_pool(name="acts", bufs=1) as apool,
            tc.tile_pool(name="pp", bufs=3) as ppool,        # P (exp out)
            tc.tile_pool(name="oqp", bufs=2) as oqpool,      # O [q, dd] staging
            tc.tile_pool(name="recp", bufs=2) as recpool,    # reciprocals
            tc.tile_pool(name="outs", bufs=8) as outpool,    # out staging
            tc.tile_pool(name="pssc", bufs=2, space="PSUM") as scpool,   # 4 banks
            tc.tile_pool(name="pspv", bufs=1, space="PSUM") as pvpool,   # 2 banks
            tc.tile_pool(name="pspj", bufs=1, space="PSUM") as pjpool,   # 1 bank
            tc.tile_pool(name="pstp", bufs=1, space="PSUM") as tppool,   # 1 bank
        ):
            # ---------------- constants / inputs -----------------------
            xt = cpool.tile([P, ET, S], dt.bfloat16)
            wq = cpool.tile([P, ET, 512], dt.bfloat16)
            wk = cpool.tile([P, ET, 512], dt.bfloat16)
            wv = cpool.tile([P, ET, 512], dt.bfloat16)
            wo = cpool.tile([P, DT, E], dt.bfloat16)
            bq = cpool.tile([P, DT], dt.float32)
            bk = cpool.tile([P, DT], dt.float32)
            bv = cpool.tile([1, 512], dt.bfloat16)
            bo = cpool.tile([P, ET], dt.float32)
            iden = cpool.tile([P, P], dt.bfloat16)
            ones1 = cpool.tile([1, P], dt.bfloat16)

            # activations
            qt8 = apool.tile([P, DT, S], dt.float8e4)   # Q^T (bias+fp8)
            kt8 = apool.tile([P, DT, S], dt.float8e4)   # K^T (bias+fp8)
            va8h = apool.tile([P, ST, NH, 72], dt.float8e4)  # fp8(V) | ones
            va8l = apool.tile([P, ST, NH, 72], dt.float8e4)  # V - fp8(V) | zeros
            scb = apool.tile([P, DT, S], dt.bfloat16)   # O^T (normalized)

            # Long-lived PSUM tiles.  Dependency tracking is TILE-granular,
            # so the scores ring is TWO alternating tiles: exp(step) then
            # only depends on its own tile's scores, and scores(step+2)
            # (same tile) WAR-waits exp(step) -- the other tile streams
            # freely underneath.
            rings = [
                scpool.tile([P, 2, 512], dt.float32, tag="sc", name="ringA"),
                scpool.tile([P, 2, 512], dt.float32, tag="sc", name="ringB"),
            ]
            pv = pvpool.tile([P, 8, P], dt.float32, tag="pv", name="pv")

            # DMA transfers serialize on the global DMA device, so order by
            # criticality: d-tile-0 K/Q slices and head-0 V cols first, the
            # full x^T (bandwidth floor ~12.6us), then everything else.
            wkr = wk_d.rearrange("(eo p) c -> p eo c", p=P)
            wqr = wq_d.rearrange("(eo p) c -> p eo c", p=P)
            wvr = wv_d.rearrange("(eo p) c -> p eo c", p=P)
            nc.sync.dma_start(wk[:, :, 0:P], wkr[:, :, 0:P])
            nc.sync.dma_start(wq[:, :, 0:P], wqr[:, :, 0:P])
            for e in range(ET):
                nc.sync.dma_start(xt[:, e, :], xt_d[e * P : (e + 1) * P, :])
            nc.sync.dma_start(bk[:], bk_d)
            nc.sync.dma_start(bq[:], bq_d)
            nc.sync.dma_start(wv[:, :, 0:HD], wvr[:, :, 0:HD])
            nc.sync.dma_start(bv[:], bv_d)
            nc.sync.dma_start(wv[:, :, HD:512], wvr[:, :, HD:512])
            nc.sync.dma_start(wk[:, :, P:512], wkr[:, :, P:512])
            nc.sync.dma_start(wq[:, :, P:512], wqr[:, :, P:512])
            nc.sync.dma_start(iden[:], iden_d)
            nc.sync.dma_start(wo[:], wo_d.rearrange("(eo p) c -> p eo c", p=P))
            nc.sync.dma_start(bo[:], bo_d)
            nc.gpsimd.memset(ones1[:], 1.0)
            nc.gpsimd.memset(va8h[:, :, :, HD : HD + 1], 1.0)
            nc.gpsimd.memset(va8l[:, :, :, HD : HD + 1], 0.0)

            # ---------------- small-chunk emitters ----------------------
            # kq/outproj chunks come as (partA, partB) sharing one psum tile;
            # with a single-buffer proj pool they are also adjacent-safe.

            def kq_halves(which, t, c):
                w_, b_, dst = (wk, bk, kt8) if which == "k" else (wq, bq, qt8)
                holder = {}

                def part0():
                    ps = pjpool.tile(
                        [P, 512], dt.float32, tag="pj", name=f"{which}{t}{c}"
                    )
                    holder[0] = ps
                    for e in range(4):
                        nc.tensor.matmul(
                            ps[:],
                            w_[:, e, t * P : (t + 1) * P],
                            xt[:, e, c * 512 : (c + 1) * 512],
                            start=(e == 0),
                            stop=False,
                        )

                def part1():
                    ps = holder[0]
                    for e in range(4, ET):
                        nc.tensor.matmul(
                            ps[:],
                            w_[:, e, t * P : (t + 1) * P],
                            xt[:, e, c * 512 : (c + 1) * 512],
                            start=False,
                            stop=(e == ET - 1),
                        )
                    nc.vector.tensor_scalar_add(
                        dst[:, t, c * 512 : (c + 1) * 512], ps[:], b_[:, t : t + 1]
                    )

                return [part0, part1]

            def v_chunk(h, st):
                """V rows for (head h, key tile st): [128 keys, 64] + bias.
                Uses the transpose-psum bank (tiny tiles) to stay off the
                kq/outproj pipeline."""
                def go():
                    ps = tppool.tile([P, P], dt.float32, tag="vps", name=f"v{h}{st}")
                    for e in range(ET):
                        nc.tensor.matmul(
                            ps[:, 0:HD],
                            xt[:, e, st * P : (st + 1) * P],
                            wv[:, e, h * HD : (h + 1) * HD],
                            start=(e == 0),
                            stop=False,
                        )
                    nc.tensor.matmul(
                        ps[:, 0:HD],
                        ones1[0:1, :],
                        bv[0:1, h * HD : (h + 1) * HD],
                        start=False,
                        stop=True,
                    )
                    nc.vector.tensor_copy(va8h[:, st, h, 0:HD], ps[:, 0:HD])
                    nc.vector.tensor_tensor(
                        va8l[:, st, h, 0:HD], ps[:, 0:HD], va8h[:, st, h, 0:HD],
                        mybir.AluOpType.subtract,
                    )
                return go

            def outproj_halves(eo, qq):
                """Partial out^T tile [128 Eo, 512 q] for query quarter qq."""
                holder = {}
                q0 = qq * 512

                def part0():
                    ps = pjpool.tile([P, 512], dt.float32, tag="pj", name=f"o{eo}{qq}")
                    holder[0] = ps
                    for t in (0, 1):
                        nc.tensor.matmul(
                            ps[:],
                            wo[:, t, eo * P : (eo + 1) * P],
                            scb[:, t, q0 : q0 + 512],
                            start=(t == 0),
                            stop=False,
                        )

                def part1():
                    ps = holder[0]
                    for t in (2, 3):
                        nc.tensor.matmul(
                            ps[:],
                            wo[:, t, eo * P : (eo + 1) * P],
                            scb[:, t, q0 : q0 + 512],
                            start=False,
                            stop=(t == DT - 1),
                        )
                    ot = outpool.tile(
                        [P, 512], dt.bfloat16, tag="ot", name=f"oe{eo}{qq}"
                    )
                    nc.vector.tensor_scalar_add(ot[:], ps[:], bo[:, eo : eo + 1])
                    nc.sync.dma_start(
                        out_d[eo * P : (eo + 1) * P, q0 : q0 + 512], ot[:]
                    )

                return [part0, part1]

            oq_tiles = {}

            def transpose_item(t, qh, qt):
                """oq [q, dd of pair t] -> scb[:, t, ...] via PE transpose.
                Uses the same psum bank as v_chunk (different tag would
                double-book the bank, so share tag/shape via bitcast)."""
                def go():
                    tp = tppool.tile([P, P], dt.float32, tag="vps", name=f"tp{t}{qh}{qt}")
                    tpb = tp[:, 0:HD].bitcast(dt.bfloat16)
                    nc.tensor.transpose(tpb, oq_tiles[(t, qh)][:, qt, :], iden[:])
                    q0 = qh * QH + qt * P
                    nc.vector.tensor_copy(scb[:, t, q0 : q0 + P], tpb)
                return go

            def transpose_batch(t, qh):
                """Tail variant: all 8 transposes into the idle pv banks,
                then the 8 copies -- avoids the 1-buffer ping-pong."""
                def go():
                    views = []
                    for qt in range(8):
                        v = pv[:, qt, 0:HD].bitcast(dt.bfloat16)
                        nc.tensor.transpose(v, oq_tiles[(t, qh)][:, qt, :], iden[:])
                        views.append(v)
                    q0 = qh * QH
                    for qt in range(8):
                        nc.vector.tensor_copy(
                            scb[:, t, q0 + qt * P : q0 + (qt + 1) * P], views[qt]
                        )
                return go

            # ---------------- attention stream ---------------------------
            # One global stream of 256 j-steps (16 phases x 16 key tiles).
            # Per step: exp(step) on ACT, then on PE scores(step+2) (its ring
            # slots were just freed by exp(step)), then the PV wave of step,
            # then interleaved thunks.  This keeps the serial chain between
            # consecutive exps down to one sem hop, across phase boundaries
            # included.
            def scores(step):
                h, qh, j = phase_of(step)
                t, hp = h // 2, (h % 2) * HD
                q0 = qh * QH
                rg = rings[step % 2]
                for qc in range(2):
                    nc.tensor.matmul(
                        rg[:, qc, :],
                        kt8[hp : hp + HD, t, j * P : (j + 1) * P]
                        .unsqueeze(1)
                        .broadcast_to((HD, 2, P)),
                        qt8[hp : hp + HD, t, q0 + qc * 512 : q0 + (qc + 1) * 512]
                        .unsqueeze(1)
                        .broadcast_to((HD, 2, 512)),
                        start=True,
                        stop=True,
                        perf_mode=DR,
                    )

            def phase_of(step):
                phase, j = divmod(step, ST)
                qh, h = divmod(phase, NH)
                return h, qh, j

            def emit_evac(h, qh):
                t, half = h // 2, h % 2
                if half == 0:
                    oq_tiles[(t, qh)] = oqpool.tile(
                        [P, 8, P], dt.bfloat16, tag="oq", name=f"oq{t}{qh}"
                    )
                oq = oq_tiles[(t, qh)]
                # one fast copy releases the pv tile for the next phase's
                # matmuls; normalize off the copy (off the critical path)
                pvc = recpool.tile([P, 8, HD + 1], dt.float32, tag="pvc", name=f"pc{h}{qh}")
                nc.vector.tensor_copy(pvc[:], pv[:, :, 0 : HD + 1])
                rec = recpool.tile([P, 8], dt.float32, tag="rec", name=f"rc{h}{qh}")
                scr = recpool.tile([P, 8], dt.float32, tag="scr", name=f"sr{h}{qh}")
                nc.vector.reciprocal_approx_accurate(
                    rec[:], pvc[:, :, HD : HD + 1].rearrange("p a b -> p (a b)"), scr[:]
                )
                for qt in range(8):
                    nc.vector.tensor_scalar(
                        oq[:, qt, half * HD : (half + 1) * HD],
                        pvc[:, qt, 0:HD],
                        rec[:, qt : qt + 1],
                        None,
                        op0=mybir.AluOpType.mult,
                    )

            def run_stream(work):
                """work: list of dicts {release, due, fns: [(fn, cost), ...]}.
                Per step, spend ~STEP_BUDGET ns of PE time on the earliest-due
                released items; a partially-emitted item always continues
                first (its parts share one psum tile)."""
                STEP_BUDGET = 620.0
                n_steps = 16 * NH * 2
                scores(0)
                scores(1)
                pending = sorted(work, key=lambda w: (w["due"], w["release"]))
                current = None
                ppair = None
                for step in range(n_steps):
                    h, qh, j = phase_of(step)
                    if step % 2 == 0:
                        ppair = ppool.tile(
                            [P, 2, 1024], dt.float8e4, tag="p", name=f"p{step}"
                        )
                    nc.scalar.activation(
                        ppair[:, step % 2, :],
                        rings[step % 2].rearrange("p a b -> p (a b)"),
                        mybir.ActivationFunctionType.Exp,
                        scale=SCALE / 2.0,
                    )
                    if step + 2 < n_steps:
                        scores(step + 2)
                    if step % 2 == 1:
                        # PV for the (j-1, j) pair: fp8 DoubleRow, V hi+lo
                        for qt in range(8):
                            lhsT = ppair[:, :, qt * P : (qt + 1) * P]
                            for vi, va8 in enumerate((va8h, va8l)):
                                nc.tensor.matmul(
                                    pv[:, qt, 0 : HD + 1],
                                    lhsT,
                                    va8[:, j - 1 : j + 1, h, 0 : HD + 1],
                                    start=(j == 1 and qt % 4 == 0 and vi == 0),
                                    stop=(j == ST - 1 and vi == 1),
                                    perf_mode=DR,
                                    skip_group_check=True,
                                )
                    budget = STEP_BUDGET
                    while budget > 0:
                        if current is None:
                            cand = [w for w in pending if w["release"] <= step]
                            if not cand:
                                break
                            current = cand[0]
                            pending.remove(current)
                            assert current["due"] >= step, (
                                f"work item overdue: emitted step {step}, "
                                f"due {current['due']}"
                            )
                        fn, cost = current["fns"].pop(0)
                        fn()
                        budget -= cost
                        if not current["fns"]:
                            current = None
                    if j == ST - 1:
                        emit_evac(h, qh)
                # anything left (tail work: release >= n_steps)
                leftovers = ([current] if current else []) + pending
                leftovers.sort(key=lambda w: (w["release"], w["due"]))
                for w in leftovers:
                    for fn, _ in w["fns"]:
                        fn()

            # ---------------- emission schedule -------------------------
            # Pre-attention prefix: K/Q d-tile 0 for the first half
            # accumulate in the four (still idle) ring slots in parallel,
            # e-interleaved so each MM fires as its x^T e-tile DMA lands.
            pref = [
                ("k", 0, rings[0], 0), ("q", 0, rings[0], 1),
                ("k", 1, rings[1], 0), ("q", 1, rings[1], 1),
            ]
            for e in range(ET):
                for which, c, rg, slot in pref:
                    w_ = wk if which == "k" else wq
                    nc.tensor.matmul(
                        rg[:, slot, :],
                        w_[:, e, 0:P],
                        xt[:, e, c * 512 : (c + 1) * 512],
                        start=(e == 0),
                        stop=(e == ET - 1),
                    )
            for which, c, rg, slot in (pref[0], pref[1], pref[3], pref[2]):
                b_, dst = (bk, kt8) if which == "k" else (bq, qt8)
                nc.vector.tensor_scalar_add(
                    dst[:, 0, c * 512 : (c + 1) * 512], rg[:, slot, :], b_[:, 0:1]
                )
            for st in range(6):
                v_chunk(0, st)()

            # Work items with release/due steps.  Due dates: a K/Q chunk
            # feeding scores(x) must finish by step x-3 (scores run two
            # steps ahead and precede thunks within a step); a V chunk
            # feeding PV(x) by step x-1.
            KQC, VC, TRC, OPC = 950.0, 340.0, 250.0, 650.0
            work = []

            def add(release, due, fns, cost):
                work.append(
                    {"release": release, "due": due,
                     "fns": [(f, cost) for f in fns]}
                )

            # V: head 0 tiles 6+ scheduled; other heads fully scheduled
            for st in range(6, ST):
                add(0, st - 1, [v_chunk(0, st)], VC)
            for h in range(1, NH):
                for st in range(ST):
                    add(0, 16 * h + st - 1, [v_chunk(h, st)], VC)
            # K: d-tile 0 chunks 2/3 (0/1 in the prefix), d-tiles 1..3 all
            for c in (2, 3):
                add(0, 4 * c - 3, kq_halves("k", 0, c), KQC / 2)
            for t2 in range(1, DT):
                for c in range(4):
                    add(0, 32 * t2 + 4 * c - 3, kq_halves("k", t2, c), KQC / 2)
            # Q: low half (chunks 0/1) due at (2t, qh0); high half at qh1
            for t2 in range(1, DT):
                for c in (0, 1):
                    add(0, 32 * t2 - 3, kq_halves("q", t2, c), KQC / 2)
            for t2 in range(DT):
                for c in (2, 3):
                    add(0, 128 + 32 * t2 - 3, kq_halves("q", t2, c), KQC / 2)
            # transposes: tight due dates so they spread right after their
            # pair completes (the single transpose buffer serializes
            # clumped transposes at ~450ns each)
            for t2 in range(DT):
                rel = 32 * t2 + 32
                add(rel, rel + 28 if t2 < DT - 1 else 145,
                    [transpose_item(t2, 0, qt) for qt in range(8)], TRC)
            for t2 in range(DT - 1):
                rel = 176 + 32 * t2
                add(rel, rel + 28,
                    [transpose_item(t2, 1, qt) for qt in range(8)], TRC)
            add(10**6 - 1, 10**6, [transpose_batch(DT - 1, 1)], TRC)
            # outproj quarters 0/1 during half 1; quarters 2/3 in the tail
            for eo in range(ET):
                for qq in (0, 1):
                    add(146, 254, outproj_halves(eo, qq), OPC)
            # tail outproj uses the (then idle) scores ring as rotating psum
            ci = [0]

            def tail_outproj(eo, qq):
                def go():
                    ps = rings[ci[0] % 2][:, (ci[0] // 2) % 2, :]
                    ci[0] += 1
                    q0 = qq * 512
                    for t in range(DT):
                        nc.tensor.matmul(
                            ps,
                            wo[:, t, eo * P : (eo + 1) * P],
                            scb[:, t, q0 : q0 + 512],
                            start=(t == 0),
                            stop=(t == DT - 1),
                        )
                    ot = outpool.tile(
                        [P, 512], dt.bfloat16, tag="ot", name=f"ot{eo}{qq}"
                    )
                    nc.vector.tensor_scalar_add(ot[:], ps, bo[:, eo : eo + 1])
                    nc.sync.dma_start(
                        out_d[eo * P : (eo + 1) * P, q0 : q0 + 512], ot[:]
                    )
                return go

            for eo in range(ET):
                for qq in (2, 3):
                    add(10**6, 10**6, [tail_outproj(eo, qq)], OPC)

            # Run the whole attention stream (tail work included).
            run_stream(work)

    nc.compile()
    return nc


def _prep_inputs(x, W_qkv, b_qkv, W_out, b_out):
    """Host-side sharding + layout prep. Returns per-core input maps."""
    w = W_qkv.reshape(E, H, 3, HD)
    b3 = b_qkv.reshape(H, 3, HD)
    iden = np.eye(P, dtype=np.float32).astype(_BF16)

    in_maps = []
    for core in range(N_CORES):
        b, hg = core // 2, core % 2
        hs = slice(hg * NH, (hg + 1) * NH)
        xt = np.ascontiguousarray(x[b].T).astype(_BF16)           # [E, S]
        wq = np.ascontiguousarray(w[:, hs, 0, :].reshape(E, 512)).astype(_BF16)
        wk = np.ascontiguousarray(w[:, hs, 1, :].reshape(E, 512)).astype(_BF16)
        wv = np.ascontiguousarray(w[:, hs, 2, :].reshape(E, 512)).astype(_BF16)
        wo = np.ascontiguousarray(W_out[hg * 512 : (hg + 1) * 512, :]).astype(_BF16)
        bq = np.ascontiguousarray(b3[hs, 0, :].reshape(DT, P).T).astype(np.float32)
        bk = np.ascontiguousarray(b3[hs, 1, :].reshape(DT, P).T).astype(np.float32)
        bv = np.ascontiguousarray(b3[hs, 2, :].reshape(1, 512)).astype(_BF16)
        bo = (np.ascontiguousarray(b_out.reshape(ET, P).T) * (1.0 if hg == 0 else 0.0)).astype(np.float32)
        in_maps.append(
            {
                "xt": xt,
                "wq": wq,
                "wk": wk,
                "wv": wv,
                "wo": wo,
                "bq": bq,
                "bk": bk,
                "bv": bv,
                "bo": bo,
                "iden": iden,
            }
        )
    return in_maps


def run_raw(x, W_qkv, b_qkv, W_out, b_out, trace=False, **kw):
    """Run on hardware; returns (full_output [B,S,E] f32, BassKernelResults)."""
    global _cached
    from concourse.bass_utils import run_bass_kernel_spmd

    if _cached is None:
        _cached = _build()
    nc = _cached
    in_maps = _prep_inputs(
        np.asarray(x), np.asarray(W_qkv), np.asarray(b_qkv),
        np.asarray(W_out), np.asarray(b_out),
    )
    res = run_bass_kernel_spmd(
        nc, in_maps, core_ids=list(range(N_CORES)), trace=trace, **kw
    )
    out = np.empty((B, S, E), dtype=np.float32)
    for b in range(B):
        acc = np.asarray(res.results[2 * b]["out"]).astype(np.float32) + np.asarray(
            res.results[2 * b + 1]["out"]
        ).astype(np.float32)
        out[b] = acc.T
    return out, res


def kernel(x, W_qkv, b_qkv, W_out, b_out):
    out, _ = run_raw(x, W_qkv, b_qkv, W_out, b_out, trace=False)
    return out
